# revision 1
# baseline (speedup 1.0000x reference)
"""GCN layer (out = A @ embeds, A in sorted-row COO) on 8 Trainium2 cores.

Strategy (row-partitioned SpMM):
  - Shard output rows across 8 cores (12500 rows each); each core owns the
    contiguous edge range whose destination rows fall in its slice (edge_row
    is sorted). The embeds table is replicated to every core.
  - Per core, output rows are processed in blocks of 512; the block
    accumulator lives in PSUM as outT [64 (D, partitions), 512 (rows, free)].
  - Edges are packed on the host into chunks of 128 (one SBUF partition per
    edge). Each chunk is assigned a 16-row window on a stride-7 grid within
    its block; windows are merged across cores so one SPMD program serves
    all 8 cores (data-dependent structure lives in the input arrays).
  - Per chunk: an indirect DMA gathers the 128 referenced embed rows
    (G [128, 64]); a selection matrix S [128, 16] with
    S[p, j] = edge_val[p] * (rowloc[p] == j) is built by two DVE
    tensor_tensor ops from precomputed rowloc/val arrays; one matmul
    accumulates G^T @ S into the psum window (rows on the free axis, so
    window offsets are unconstrained).
  - Blocks are flushed psum -> SBUF -> DRAM as outT [64, 12800] per core;
    the final transpose/concat happens on the host.
"""
import math
import numpy as np

WSZ = 16
WSTRIDE = 7
PAD_ROWLOC = 99.0
N_CORES = 8
RB = 512
D = 64


def _build_windows(rb):
    ws = list(range(0, rb - WSZ + 1, WSTRIDE))
    if ws[-1] != rb - WSZ:
        ws.append(rb - WSZ)
    return ws


def _pack_core(edge_row, rptr, r0, r1, rb, w_list):
    nb = math.ceil((r1 - r0) / rb)
    nwin = len(w_list)
    last_w = w_list[-1]
    grid_last = (nwin - 1) * WSTRIDE
    blocks = []
    for b in range(nb):
        br0 = r0 + b * rb
        br1 = min(br0 + rb, r1)
        win_chunks = {}
        for r in range(br0, br1):
            s, e = rptr[r], rptr[r + 1]
            if s == e:
                continue
            rl = r - br0
            jlo = max(0, -(-(rl - (WSZ - 1)) // WSTRIDE))
            jhi = min(rl // WSTRIDE, nwin - 1)
            elig = list(range(jlo, jhi + 1))
            if last_w != grid_last and last_w <= rl <= last_w + WSZ - 1:
                if not (elig and elig[-1] == nwin - 1):
                    elig.append(nwin - 1)
            elig = [j for j in elig if w_list[j] <= rl <= w_list[j] + WSZ - 1]
            rem = e - s
            pos = s
            for j in elig:
                if rem == 0:
                    break
                for ch in win_chunks.get(j, []):
                    space = 128 - ch[0]
                    if space <= 0:
                        continue
                    take = min(space, rem)
                    ch[1].append((pos, take, rl - w_list[j]))
                    ch[0] += take
                    pos += take
                    rem -= take
                    if rem == 0:
                        break
            while rem > 0:
                j = elig[-1]
                take = min(128, rem)
                win_chunks.setdefault(j, []).append(
                    [take, [(pos, take, rl - w_list[j])]]
                )
                pos += take
                rem -= take
        blocks.append(win_chunks)
    return blocks


def _prepare(edge_row, edge_col, edge_val, n_nodes):
    rpc = n_nodes // N_CORES
    nb = math.ceil(rpc / RB)
    w_list = _build_windows(RB)
    nwin = len(w_list)
    rptr = np.searchsorted(edge_row, np.arange(n_nodes + 1)).astype(np.int64)

    per_core = [
        _pack_core(edge_row, rptr, k * rpc, (k + 1) * rpc, RB, w_list)
        for k in range(N_CORES)
    ]

    nch = np.zeros((nb, nwin), np.int64)
    for k in range(N_CORES):
        for b in range(nb):
            for j, chs in per_core[k][b].items():
                nch[b, j] = max(nch[b, j], len(chs))

    block_windows = []
    block_nch = []
    for b in range(nb):
        lst = []
        for j in range(nwin):
            lst.extend([j] * int(nch[b, j]))
        block_windows.append(lst)
        block_nch.append(len(lst))
    c_off = np.concatenate([[0], np.cumsum(block_nch)]).astype(np.int64)
    totch = int(c_off[-1])

    idx = np.full((N_CORES, 128, totch), n_nodes, np.int32)
    rowloc = np.full((N_CORES, 128, totch), PAD_ROWLOC, np.float32)
    val = np.zeros((N_CORES, 128, totch), np.float32)

    for k in range(N_CORES):
        for b in range(nb):
            slot_of = {}
            cnt = {}
            for ci, j in enumerate(block_windows[b]):
                slot_of[(j, cnt.get(j, 0))] = int(c_off[b]) + ci
                cnt[j] = cnt.get(j, 0) + 1
            for j, chs in per_core[k][b].items():
                for copy, ch in enumerate(chs):
                    c = slot_of[(j, copy)]
                    p = 0
                    for (pos, take, rl) in ch[1]:
                        idx[k, p:p+take, c] = edge_col[pos:pos+take]
                        rowloc[k, p:p+take, c] = rl
                        val[k, p:p+take, c] = edge_val[pos:pos+take]
                        p += take

    maxnch = max(block_nch)
    iota = np.tile(np.arange(WSZ, dtype=np.float32), (128, maxnch))
    return dict(
        n_nodes=n_nodes, nb=nb, rpc=rpc, w_list=w_list,
        block_windows=block_windows, block_nch=block_nch,
        c_off=c_off, totch=totch, maxnch=maxnch,
        idx=idx, rowloc=rowloc, val=val, iota=iota,
    )


def _build_program(prep):
    import concourse.bacc as bacc
    import concourse.bass as bass
    import concourse.mybir as mybir
    import concourse.tile as tile

    n = prep["n_nodes"]
    nb = prep["nb"]
    totch, maxnch = prep["totch"], prep["maxnch"]
    w_list = prep["w_list"]

    nc = bacc.Bacc("TRN2", target_bir_lowering=False, debug=False)
    embeds_d = nc.dram_tensor("embeds", [n, D], mybir.dt.float32, kind="ExternalInput")
    idx_d = nc.dram_tensor("idx", [128, totch], mybir.dt.int32, kind="ExternalInput")
    rowloc_d = nc.dram_tensor("rowloc", [128, totch], mybir.dt.float32, kind="ExternalInput")
    val_d = nc.dram_tensor("val", [128, totch], mybir.dt.float32, kind="ExternalInput")
    iota_d = nc.dram_tensor("iota", [128, maxnch * WSZ], mybir.dt.float32, kind="ExternalInput")
    outT_d = nc.dram_tensor("outT", [D, nb * RB], mybir.dt.float32, kind="ExternalOutput")

    with tile.TileContext(nc) as tc:
        with (
            tc.tile_pool(name="const", bufs=1) as constp,
            tc.tile_pool(name="gp", bufs=2) as gp,
            tc.tile_pool(name="sp", bufs=2) as sp,
            tc.tile_pool(name="stp", bufs=2) as stp,
            tc.tile_pool(name="pp", bufs=2, space="PSUM") as pp,
        ):
            idx_t = constp.tile([128, totch], mybir.dt.int32)
            rowloc_t = constp.tile([128, totch], mybir.dt.float32)
            val_t = constp.tile([128, totch], mybir.dt.float32)
            iota_t = constp.tile([128, maxnch * WSZ], mybir.dt.float32)
            zero64_t = constp.tile([128, D], mybir.dt.float32)
            zrhs_t = constp.tile([128, RB], mybir.dt.float32)

            nc.sync.dma_start(idx_t[:], idx_d[:])
            nc.sync.dma_start(rowloc_t[:], rowloc_d[:])
            nc.sync.dma_start(val_t[:], val_d[:])
            nc.sync.dma_start(iota_t[:], iota_d[:])
            nc.vector.memset(zero64_t[:], 0.0)
            nc.vector.memset(zrhs_t[:], 0.0)

            bc_reg = nc.gpsimd.to_reg(n - 1)

            for b in range(nb):
                c0 = int(prep["c_off"][b])
                nch = prep["block_nch"][b]
                if nch == 0:
                    continue
                G_t = gp.tile([128, nch, D], mybir.dt.float32, tag="G")
                for g0 in range(nch):
                    nc.gpsimd.indirect_dma_start(
                        out=G_t[:, g0, :],
                        out_offset=None,
                        in_=embeds_d[:],
                        in_offset=bass.IndirectOffsetOnAxis(
                            ap=idx_t[:, c0 + g0:c0 + g0 + 1], axis=0
                        ),
                        bounds_check=bc_reg,
                        oob_is_err=False,
                    )
                S01_t = sp.tile([128, nch * WSZ], mybir.dt.float32, tag="S01")
                S_t = sp.tile([128, nch * WSZ], mybir.dt.float32, tag="S")
                rl_b = rowloc_t[:, c0:c0 + nch, None].to_broadcast([128, nch, WSZ])
                vl_b = val_t[:, c0:c0 + nch, None].to_broadcast([128, nch, WSZ])
                iota3 = iota_t[:, :nch * WSZ].rearrange("p (c j) -> p c j", c=nch)
                S01_3 = S01_t[:].rearrange("p (c j) -> p c j", c=nch)
                S_3 = S_t[:].rearrange("p (c j) -> p c j", c=nch)
                nc.vector.tensor_tensor(S01_3, iota3, rl_b, mybir.AluOpType.is_equal)
                nc.vector.tensor_tensor(S_3, S01_3, vl_b, mybir.AluOpType.mult)

                psum_t = pp.tile([D, RB], mybir.dt.float32)
                nc.tensor.matmul(
                    out=psum_t[:], lhsT=zero64_t[:], rhs=zrhs_t[:],
                    start=True, stop=False,
                )
                for ci in range(nch):
                    w = w_list[prep["block_windows"][b][ci]]
                    nc.tensor.matmul(
                        out=psum_t[:, w:w + WSZ],
                        lhsT=G_t[:, ci, :],
                        rhs=S_t[:, ci * WSZ:(ci + 1) * WSZ],
                        start=False,
                        stop=(ci == nch - 1),
                    )
                stage_t = stp.tile([D, RB], mybir.dt.float32, tag="stage")
                nc.any.tensor_copy(out=stage_t[:], in_=psum_t[:])
                nc.sync.dma_start(outT_d[:, b * RB:(b + 1) * RB], stage_t[:])

    nc.finalize()
    return nc


def kernel(edge_row, edge_col, edge_val, embeds, num_nodes):
    from concourse.bass_utils import run_bass_kernel_spmd

    n = int(num_nodes)
    edge_row = np.asarray(edge_row)
    edge_col = np.asarray(edge_col)
    edge_val = np.asarray(edge_val)
    embeds = np.ascontiguousarray(np.asarray(embeds, dtype=np.float32))

    prep = _prepare(edge_row, edge_col, edge_val, n)
    nc = _build_program(prep)

    in_maps = [
        dict(
            embeds=embeds,
            idx=prep["idx"][k],
            rowloc=prep["rowloc"][k],
            val=prep["val"][k],
            iota=prep["iota"],
        )
        for k in range(N_CORES)
    ]
    res = run_bass_kernel_spmd(nc, in_maps, list(range(N_CORES)))

    rpc = prep["rpc"]
    out = np.zeros((n, D), np.float32)
    for k in range(N_CORES):
        outT = res.results[k]["outT"]
        out[k * rpc:(k + 1) * rpc] = outT[:, :rpc].T
    return out



# revision 2
# speedup vs baseline: 6.8711x; 6.8711x over previous
"""GCN layer (out = A @ embeds, A in sorted-row COO) on 8 Trainium2 cores.

Strategy (row-partitioned SpMM, wire-optimized):
  - Shard output rows across 8 cores (12500 rows each); each core owns the
    contiguous edge range whose destination rows fall in its slice (edge_row
    is sorted).
  - The embeds table is NOT replicated over the (slow) host link: each core
    receives only its 1/8 row-shard in fp16 and the full table is rebuilt
    on-device with an HBM AllGather over NeuronLink.
  - Each edge travels as ONE int32: col index (17 bits) | row offset within
    its 16-row window (4 bits) | value quantized to 11 bits. The value scale
    is folded into the fp16 embeds on the host. Unpacking (shift/and,
    int->fp16 convert) happens on-device on the DVE.
  - Per core, output rows are processed in blocks of 512; the block
    accumulator lives in PSUM as outT [64 (D, partitions), 512 (rows, free)].
  - Edges are packed on the host into chunks of 128 (one SBUF partition per
    edge). Each chunk is assigned a 16-row window on a stride-7 grid within
    its block; windows are merged across cores so one SPMD program serves
    all 8 cores (data-dependent structure lives in the input arrays).
  - Per chunk: an indirect DMA gathers the 128 referenced fp16 embed rows
    (G [128, 64]); a selection matrix S [128, 16] with
    S[p, j] = val[p] * (rowloc[p] == j) is built by two DVE tensor_tensor
    ops against an on-device iota; one fp16 matmul accumulates G^T @ S into
    the psum window (rows on the free axis, so window offsets are
    unconstrained).
  - Blocks are flushed psum -> SBUF (fp16) -> DRAM as outT [64, 12500] per
    core; the final transpose/concat happens on the host.
"""
import math
import numpy as np

WSZ = 16
WSTRIDE = 7
N_CORES = 8
RB = 512
D = 64
VQ_MAX = 2047  # 11-bit value quantization


def _build_windows(rb):
    ws = list(range(0, rb - WSZ + 1, WSTRIDE))
    if ws[-1] != rb - WSZ:
        ws.append(rb - WSZ)
    return ws


def _pack_core(edge_row, rptr, r0, r1, rb, w_list):
    nb = math.ceil((r1 - r0) / rb)
    nwin = len(w_list)
    last_w = w_list[-1]
    grid_last = (nwin - 1) * WSTRIDE
    blocks = []
    for b in range(nb):
        br0 = r0 + b * rb
        br1 = min(br0 + rb, r1)
        win_chunks = {}
        for r in range(br0, br1):
            s, e = rptr[r], rptr[r + 1]
            if s == e:
                continue
            rl = r - br0
            jlo = max(0, -(-(rl - (WSZ - 1)) // WSTRIDE))
            jhi = min(rl // WSTRIDE, nwin - 1)
            elig = list(range(jlo, jhi + 1))
            if last_w != grid_last and last_w <= rl <= last_w + WSZ - 1:
                if not (elig and elig[-1] == nwin - 1):
                    elig.append(nwin - 1)
            elig = [j for j in elig if w_list[j] <= rl <= w_list[j] + WSZ - 1]
            rem = e - s
            pos = s
            for j in elig:
                if rem == 0:
                    break
                for ch in win_chunks.get(j, []):
                    space = 128 - ch[0]
                    if space <= 0:
                        continue
                    take = min(space, rem)
                    ch[1].append((pos, take, rl - w_list[j]))
                    ch[0] += take
                    pos += take
                    rem -= take
                    if rem == 0:
                        break
            while rem > 0:
                j = elig[-1]
                take = min(128, rem)
                win_chunks.setdefault(j, []).append(
                    [take, [(pos, take, rl - w_list[j])]]
                )
                pos += take
                rem -= take
        blocks.append(win_chunks)
    return blocks


def _prepare(edge_row, edge_col, edge_val, n_nodes):
    assert n_nodes < (1 << 17) and n_nodes % N_CORES == 0
    rpc = n_nodes // N_CORES
    nb = math.ceil(rpc / RB)
    w_list = _build_windows(RB)
    nwin = len(w_list)
    rptr = np.searchsorted(edge_row, np.arange(n_nodes + 1)).astype(np.int64)

    per_core = [
        _pack_core(edge_row, rptr, k * rpc, (k + 1) * rpc, RB, w_list)
        for k in range(N_CORES)
    ]

    nch = np.zeros((nb, nwin), np.int64)
    for k in range(N_CORES):
        for b in range(nb):
            for j, chs in per_core[k][b].items():
                nch[b, j] = max(nch[b, j], len(chs))

    block_windows = []
    block_nch = []
    for b in range(nb):
        lst = []
        for j in range(nwin):
            lst.extend([j] * int(nch[b, j]))
        block_windows.append(lst)
        block_nch.append(len(lst))
    c_off = np.concatenate([[0], np.cumsum(block_nch)]).astype(np.int64)
    totch = int(c_off[-1])

    # quantize values to 11 bits; the scale is folded into the fp16 embeds
    vscale = float(np.max(edge_val)) if edge_val.size else 1.0
    vscale = max(vscale, 1e-30)
    vq_all = np.clip(
        np.rint(np.asarray(edge_val, np.float64) / vscale * VQ_MAX), 0, VQ_MAX
    ).astype(np.int64)

    # packed word: idx | rowloc << 17 | vq << 21  (pad: idx=0, rl=0, vq=0)
    packed = np.zeros((N_CORES, 128, totch), np.int32)
    for k in range(N_CORES):
        for b in range(nb):
            slot_of = {}
            cnt = {}
            for ci, j in enumerate(block_windows[b]):
                slot_of[(j, cnt.get(j, 0))] = int(c_off[b]) + ci
                cnt[j] = cnt.get(j, 0) + 1
            for j, chs in per_core[k][b].items():
                for copy, ch in enumerate(chs):
                    c = slot_of[(j, copy)]
                    p = 0
                    for (pos, take, rl) in ch[1]:
                        word = (
                            edge_col[pos:pos + take].astype(np.int64)
                            | (rl << 17)
                            | (vq_all[pos:pos + take] << 21)
                        )
                        packed[k, p:p + take, c] = word.astype(np.int32)
                        p += take

    maxnch = max(block_nch)
    return dict(
        n_nodes=n_nodes, nb=nb, rpc=rpc, w_list=w_list,
        block_windows=block_windows, block_nch=block_nch,
        c_off=c_off, totch=totch, maxnch=maxnch,
        packed=packed, vscale=vscale,
    )


def _build_program(prep):
    import concourse.bacc as bacc
    import concourse.bass as bass
    import concourse.mybir as mybir
    import concourse.tile as tile

    n = prep["n_nodes"]
    nb = prep["nb"]
    rpc = prep["rpc"]
    nsh = n // N_CORES
    totch, maxnch = prep["totch"], prep["maxnch"]
    w_list = prep["w_list"]

    nc = bacc.Bacc(
        "TRN2", target_bir_lowering=False, debug=False, num_devices=N_CORES
    )
    eshard_d = nc.dram_tensor(
        "eshard", [nsh, D], mybir.dt.float16, kind="ExternalInput"
    )
    packed_d = nc.dram_tensor(
        "packed", [128, totch], mybir.dt.int32, kind="ExternalInput"
    )
    outT_d = nc.dram_tensor("outT", [D, rpc], mybir.dt.float16, kind="ExternalOutput")

    with tile.TileContext(nc) as tc:
        with (
            tc.tile_pool(name="dram", bufs=1, space="DRAM") as dram,
            tc.tile_pool(name="const", bufs=1) as constp,
            tc.tile_pool(name="gp", bufs=2) as gp,
            tc.tile_pool(name="sp", bufs=2) as sp,
            tc.tile_pool(name="stp", bufs=2) as stp,
            tc.tile_pool(name="pp", bufs=2, space="PSUM") as pp,
        ):
            bounce = dram.tile([nsh, D], mybir.dt.float16)
            gathered = dram.tile([n, D], mybir.dt.float16)
            nc.gpsimd.dma_start(bounce[:], eshard_d[:])
            nc.gpsimd.collective_compute(
                "AllGather",
                mybir.AluOpType.bypass,
                replica_groups=[list(range(N_CORES))],
                ins=[bounce.opt()],
                outs=[gathered.opt()],
            )

            packed_t = constp.tile([128, totch], mybir.dt.int32)
            nc.sync.dma_start(packed_t[:], packed_d[:])
            idx_t = constp.tile([128, totch], mybir.dt.int32)
            rl_h = constp.tile([128, totch], mybir.dt.float16)
            vs_h = constp.tile([128, totch], mybir.dt.float16)
            tmp_i = constp.tile([128, totch], mybir.dt.int32)
            nc.vector.tensor_scalar(
                idx_t[:], packed_t[:], 0x1FFFF, None, mybir.AluOpType.bitwise_and
            )
            nc.vector.tensor_scalar(
                tmp_i[:], packed_t[:], 17, 0xF,
                mybir.AluOpType.logical_shift_right, mybir.AluOpType.bitwise_and,
            )
            nc.any.tensor_copy(out=rl_h[:], in_=tmp_i[:])
            nc.vector.tensor_scalar(
                tmp_i[:], packed_t[:], 21, None, mybir.AluOpType.logical_shift_right
            )
            nc.any.tensor_copy(out=vs_h[:], in_=tmp_i[:])
            nc.vector.tensor_scalar(
                vs_h[:], vs_h[:], 1.0 / VQ_MAX, None, mybir.AluOpType.mult
            )

            iota_i = constp.tile([128, maxnch * WSZ], mybir.dt.int32)
            nc.gpsimd.iota(iota_i[:], [[0, maxnch], [1, WSZ]], channel_multiplier=0)
            iota_h = constp.tile([128, maxnch * WSZ], mybir.dt.float16)
            nc.any.tensor_copy(out=iota_h[:], in_=iota_i[:])

            zero64_t = constp.tile([128, D], mybir.dt.float16)
            zrhs_t = constp.tile([128, RB], mybir.dt.float16)
            nc.vector.memset(zero64_t[:], 0.0)
            nc.vector.memset(zrhs_t[:], 0.0)

            for b in range(nb):
                c0 = int(prep["c_off"][b])
                nch = prep["block_nch"][b]
                if nch == 0:
                    continue
                G_t = gp.tile([128, nch, D], mybir.dt.float16, tag="G")
                for g0 in range(nch):
                    nc.gpsimd.indirect_dma_start(
                        out=G_t[:, g0, :],
                        out_offset=None,
                        in_=gathered[:],
                        in_offset=bass.IndirectOffsetOnAxis(
                            ap=idx_t[:, c0 + g0:c0 + g0 + 1], axis=0
                        ),
                    )
                S01_t = sp.tile([128, nch * WSZ], mybir.dt.float16, tag="S01")
                S_t = sp.tile([128, nch * WSZ], mybir.dt.float16, tag="S")
                rl_b = rl_h[:, c0:c0 + nch, None].to_broadcast([128, nch, WSZ])
                vl_b = vs_h[:, c0:c0 + nch, None].to_broadcast([128, nch, WSZ])
                iota3 = iota_h[:, :nch * WSZ].rearrange("p (c j) -> p c j", c=nch)
                S01_3 = S01_t[:].rearrange("p (c j) -> p c j", c=nch)
                S_3 = S_t[:].rearrange("p (c j) -> p c j", c=nch)
                nc.vector.tensor_tensor(S01_3, iota3, rl_b, mybir.AluOpType.is_equal)
                nc.vector.tensor_tensor(S_3, S01_3, vl_b, mybir.AluOpType.mult)

                psum_t = pp.tile([D, RB], mybir.dt.float32)
                nc.tensor.matmul(
                    out=psum_t[:], lhsT=zero64_t[:], rhs=zrhs_t[:],
                    start=True, stop=False,
                )
                for ci in range(nch):
                    w = w_list[prep["block_windows"][b][ci]]
                    nc.tensor.matmul(
                        out=psum_t[:, w:w + WSZ],
                        lhsT=G_t[:, ci, :],
                        rhs=S_t[:, ci * WSZ:(ci + 1) * WSZ],
                        start=False,
                        stop=(ci == nch - 1),
                    )
                cw = min(RB, rpc - b * RB)  # last block is partial
                stage_t = stp.tile([D, RB], mybir.dt.float16, tag="stage")
                nc.any.tensor_copy(out=stage_t[:], in_=psum_t[:])
                nc.sync.dma_start(
                    outT_d[:, b * RB:b * RB + cw], stage_t[:, :cw]
                )

    nc.finalize()
    return nc


def _make_in_maps(prep, embeds):
    n = prep["n_nodes"]
    nsh = n // N_CORES
    esc = (np.asarray(embeds, np.float32) * prep["vscale"]).astype(np.float16)
    return [
        dict(
            eshard=np.ascontiguousarray(esc[k * nsh:(k + 1) * nsh]),
            packed=prep["packed"][k],
        )
        for k in range(N_CORES)
    ]


def kernel(edge_row, edge_col, edge_val, embeds, num_nodes):
    from concourse.bass_utils import run_bass_kernel_spmd

    n = int(num_nodes)
    edge_row = np.asarray(edge_row)
    edge_col = np.asarray(edge_col)
    edge_val = np.asarray(edge_val)

    prep = _prepare(edge_row, edge_col, edge_val, n)
    nc = _build_program(prep)
    in_maps = _make_in_maps(prep, embeds)
    res = run_bass_kernel_spmd(nc, in_maps, list(range(N_CORES)))

    rpc = prep["rpc"]
    out = np.empty((n, D), np.float32)
    for k in range(N_CORES):
        outT = res.results[k]["outT"]  # [D, rpc] fp16
        out[k * rpc:(k + 1) * rpc] = outT.astype(np.float32).T
    return out


# revision 6
# speedup vs baseline: 15.3929x; 2.2402x over previous
"""GCN layer (out = A @ embeds, A in sorted-row COO) on 8 Trainium2 cores.

Strategy (row-partitioned SpMM, wire-optimized):
  - Shard output rows across 8 cores (12500 rows each); each core owns the
    contiguous edge range whose destination rows fall in its slice (edge_row
    is sorted).
  - The embeds table is NOT replicated over the (slow) host link: each core
    receives only its 1/8 row-shard in fp16 and the full table is rebuilt
    on-device with an HBM AllGather over NeuronLink.
  - Each edge travels as ONE int32: col index (17 bits) | row offset within
    its 16-row window (4 bits) | value quantized to 11 bits. The value scale
    is folded into the fp16 embeds on the host. Unpacking (shift/and,
    int->fp16 convert) happens on-device on the DVE.
  - Per core, output rows are processed in blocks of 512; the block
    accumulator lives in PSUM as outT [64 (D, partitions), 512 (rows, free)].
  - Edges are packed on the host into chunks of 128 (one SBUF partition per
    edge). Each chunk is assigned a 16-row window on a stride-7 grid within
    its block; windows are merged across cores so one SPMD program serves
    all 8 cores (data-dependent structure lives in the input arrays).
  - Per chunk: an indirect DMA gathers the 128 referenced fp16 embed rows
    (G [128, 64]); a selection matrix S [128, 16] with
    S[p, j] = val[p] * (rowloc[p] == j) is built by two DVE tensor_tensor
    ops against an on-device iota; one fp16 matmul accumulates G^T @ S into
    the psum window (rows on the free axis, so window offsets are
    unconstrained).
  - Blocks are flushed psum -> SBUF (fp16) -> DRAM as outT [64, 12500] per
    core; the final transpose/concat happens on the host.
"""
import math
import numpy as np

WSZ = 16
WSTRIDE = 7
N_CORES = 8
RB = 512
D = 64
VQ_MAX = 2047  # 11-bit value quantization


def _build_windows(rb):
    ws = list(range(0, rb - WSZ + 1, WSTRIDE))
    if ws[-1] != rb - WSZ:
        ws.append(rb - WSZ)
    return ws


def _pack_core(edge_row, rptr, r0, r1, rb, w_list):
    nb = math.ceil((r1 - r0) / rb)
    nwin = len(w_list)
    last_w = w_list[-1]
    grid_last = (nwin - 1) * WSTRIDE
    blocks = []
    for b in range(nb):
        br0 = r0 + b * rb
        br1 = min(br0 + rb, r1)
        win_chunks = {}
        for r in range(br0, br1):
            s, e = rptr[r], rptr[r + 1]
            if s == e:
                continue
            rl = r - br0
            jlo = max(0, -(-(rl - (WSZ - 1)) // WSTRIDE))
            jhi = min(rl // WSTRIDE, nwin - 1)
            elig = list(range(jlo, jhi + 1))
            if last_w != grid_last and last_w <= rl <= last_w + WSZ - 1:
                if not (elig and elig[-1] == nwin - 1):
                    elig.append(nwin - 1)
            elig = [j for j in elig if w_list[j] <= rl <= w_list[j] + WSZ - 1]
            rem = e - s
            pos = s
            for j in elig:
                if rem == 0:
                    break
                for ch in win_chunks.get(j, []):
                    space = 128 - ch[0]
                    if space <= 0:
                        continue
                    take = min(space, rem)
                    ch[1].append((pos, take, rl - w_list[j]))
                    ch[0] += take
                    pos += take
                    rem -= take
                    if rem == 0:
                        break
            while rem > 0:
                j = elig[-1]
                take = min(128, rem)
                win_chunks.setdefault(j, []).append(
                    [take, [(pos, take, rl - w_list[j])]]
                )
                pos += take
                rem -= take
        blocks.append(win_chunks)
    return blocks


def _prepare(edge_row, edge_col, edge_val, n_nodes):
    assert n_nodes < (1 << 17) and n_nodes % N_CORES == 0
    rpc = n_nodes // N_CORES
    nb = math.ceil(rpc / RB)
    w_list = _build_windows(RB)
    nwin = len(w_list)
    rptr = np.searchsorted(edge_row, np.arange(n_nodes + 1)).astype(np.int64)

    per_core = [
        _pack_core(edge_row, rptr, k * rpc, (k + 1) * rpc, RB, w_list)
        for k in range(N_CORES)
    ]

    nch = np.zeros((nb, nwin), np.int64)
    for k in range(N_CORES):
        for b in range(nb):
            for j, chs in per_core[k][b].items():
                nch[b, j] = max(nch[b, j], len(chs))

    block_windows = []
    block_nch = []
    for b in range(nb):
        lst = []
        for j in range(nwin):
            lst.extend([j] * int(nch[b, j]))
        block_windows.append(lst)
        block_nch.append(len(lst))
    c_off = np.concatenate([[0], np.cumsum(block_nch)]).astype(np.int64)
    totch = int(c_off[-1])

    # quantize values to 11 bits; the scale is folded into the fp16 embeds
    vscale = float(np.max(edge_val)) if edge_val.size else 1.0
    vscale = max(vscale, 1e-30)
    vq_all = np.clip(
        np.rint(np.asarray(edge_val, np.float64) / vscale * VQ_MAX), 0, VQ_MAX
    ).astype(np.int64)

    # packed word: idx | rowloc << 17 | vq << 21  (pad: idx=0, rl=0, vq=0)
    packed = np.zeros((N_CORES, 128, totch), np.int32)
    for k in range(N_CORES):
        for b in range(nb):
            slot_of = {}
            cnt = {}
            for ci, j in enumerate(block_windows[b]):
                slot_of[(j, cnt.get(j, 0))] = int(c_off[b]) + ci
                cnt[j] = cnt.get(j, 0) + 1
            for j, chs in per_core[k][b].items():
                for copy, ch in enumerate(chs):
                    c = slot_of[(j, copy)]
                    p = 0
                    for (pos, take, rl) in ch[1]:
                        word = (
                            edge_col[pos:pos + take].astype(np.int64)
                            | (rl << 17)
                            | (vq_all[pos:pos + take] << 21)
                        )
                        packed[k, p:p + take, c] = word.astype(np.int32)
                        p += take

    maxnch = max(block_nch)
    return dict(
        n_nodes=n_nodes, nb=nb, rpc=rpc, w_list=w_list,
        block_windows=block_windows, block_nch=block_nch,
        c_off=c_off, totch=totch, maxnch=maxnch,
        packed=packed, vscale=vscale,
    )


def _build_program(prep):
    import concourse.bacc as bacc
    import concourse.bass as bass
    import concourse.mybir as mybir
    import concourse.tile as tile

    n = prep["n_nodes"]
    nb = prep["nb"]
    rpc = prep["rpc"]
    nsh = n // N_CORES
    totch, maxnch = prep["totch"], prep["maxnch"]
    w_list = prep["w_list"]

    nc = bacc.Bacc(
        "TRN2", target_bir_lowering=False, debug=False, num_devices=N_CORES
    )
    eshard_d = nc.dram_tensor(
        "eshard", [nsh, D], mybir.dt.float16, kind="ExternalInput"
    )
    packed_d = nc.dram_tensor(
        "packed", [128, totch], mybir.dt.int32, kind="ExternalInput"
    )
    outT_d = nc.dram_tensor("outT", [D, rpc], mybir.dt.float16, kind="ExternalOutput")

    with tile.TileContext(nc) as tc:
        with (
            tc.tile_pool(name="dram", bufs=1, space="DRAM") as dram,
            tc.tile_pool(name="const", bufs=1) as constp,
            tc.tile_pool(name="gp", bufs=2) as gp,
            tc.tile_pool(name="sp", bufs=2) as sp,
            tc.tile_pool(name="stp", bufs=2) as stp,
            tc.tile_pool(name="pp", bufs=2, space="PSUM") as pp,
        ):
            bounce = dram.tile([nsh, D], mybir.dt.float16)
            gathered = dram.tile([n, D], mybir.dt.float16)
            nc.gpsimd.dma_start(bounce[:], eshard_d[:])
            nc.gpsimd.collective_compute(
                "AllGather",
                mybir.AluOpType.bypass,
                replica_groups=[list(range(N_CORES))],
                ins=[bounce.opt()],
                outs=[gathered.opt()],
            )

            packed_t = constp.tile([128, totch], mybir.dt.int32)
            nc.sync.dma_start(packed_t[:], packed_d[:])
            idx_t = constp.tile([128, totch], mybir.dt.int32)
            rl_h = constp.tile([128, totch], mybir.dt.float16)
            vs_h = constp.tile([128, totch], mybir.dt.float16)
            tmp_i = constp.tile([128, totch], mybir.dt.int32)
            nc.vector.tensor_scalar(
                idx_t[:], packed_t[:], 0x1FFFF, None, mybir.AluOpType.bitwise_and
            )
            nc.vector.tensor_scalar(
                tmp_i[:], packed_t[:], 17, 0xF,
                mybir.AluOpType.logical_shift_right, mybir.AluOpType.bitwise_and,
            )
            nc.any.tensor_copy(out=rl_h[:], in_=tmp_i[:])
            nc.vector.tensor_scalar(
                tmp_i[:], packed_t[:], 21, None, mybir.AluOpType.logical_shift_right
            )
            nc.any.tensor_copy(out=vs_h[:], in_=tmp_i[:])
            nc.vector.tensor_scalar(
                vs_h[:], vs_h[:], 1.0 / VQ_MAX, None, mybir.AluOpType.mult
            )

            iota_i = constp.tile([128, maxnch * WSZ], mybir.dt.int32)
            nc.gpsimd.iota(iota_i[:], [[0, maxnch], [1, WSZ]], channel_multiplier=0)
            iota_h = constp.tile([128, maxnch * WSZ], mybir.dt.float16)
            nc.any.tensor_copy(out=iota_h[:], in_=iota_i[:])

            zero64_t = constp.tile([128, D], mybir.dt.float16)
            zrhs_t = constp.tile([128, RB], mybir.dt.float16)
            nc.vector.memset(zero64_t[:], 0.0)
            nc.vector.memset(zrhs_t[:], 0.0)

            for b in range(nb):
                c0 = int(prep["c_off"][b])
                nch = prep["block_nch"][b]
                if nch == 0:
                    continue
                G_t = gp.tile([128, nch, D], mybir.dt.float16, tag="G")
                for g0 in range(nch):
                    nc.gpsimd.indirect_dma_start(
                        out=G_t[:, g0, :],
                        out_offset=None,
                        in_=gathered[:],
                        in_offset=bass.IndirectOffsetOnAxis(
                            ap=idx_t[:, c0 + g0:c0 + g0 + 1], axis=0
                        ),
                    )
                S01_t = sp.tile([128, nch * WSZ], mybir.dt.float16, tag="S01")
                S_t = sp.tile([128, nch * WSZ], mybir.dt.float16, tag="S")
                rl_b = rl_h[:, c0:c0 + nch, None].to_broadcast([128, nch, WSZ])
                vl_b = vs_h[:, c0:c0 + nch, None].to_broadcast([128, nch, WSZ])
                iota3 = iota_h[:, :nch * WSZ].rearrange("p (c j) -> p c j", c=nch)
                S01_3 = S01_t[:].rearrange("p (c j) -> p c j", c=nch)
                S_3 = S_t[:].rearrange("p (c j) -> p c j", c=nch)
                nc.vector.tensor_tensor(S01_3, iota3, rl_b, mybir.AluOpType.is_equal)
                nc.vector.tensor_tensor(S_3, S01_3, vl_b, mybir.AluOpType.mult)

                psum_t = pp.tile([D, RB], mybir.dt.float32)
                nc.tensor.matmul(
                    out=psum_t[:], lhsT=zero64_t[:], rhs=zrhs_t[:],
                    start=True, stop=False,
                )
                for ci in range(nch):
                    w = w_list[prep["block_windows"][b][ci]]
                    nc.tensor.matmul(
                        out=psum_t[:, w:w + WSZ],
                        lhsT=G_t[:, ci, :],
                        rhs=S_t[:, ci * WSZ:(ci + 1) * WSZ],
                        start=False,
                        stop=(ci == nch - 1),
                    )
                cw = min(RB, rpc - b * RB)  # last block is partial
                stage_t = stp.tile([D, RB], mybir.dt.float16, tag="stage")
                nc.any.tensor_copy(out=stage_t[:], in_=psum_t[:])
                nc.sync.dma_start(
                    outT_d[:, b * RB:b * RB + cw], stage_t[:, :cw]
                )

    nc.finalize()
    return nc


def _make_in_maps(prep, embeds):
    n = prep["n_nodes"]
    nsh = n // N_CORES
    esc = (np.asarray(embeds, np.float32) * prep["vscale"]).astype(np.float16)
    return [
        dict(
            eshard=np.ascontiguousarray(esc[k * nsh:(k + 1) * nsh]),
            packed=prep["packed"][k],
        )
        for k in range(N_CORES)
    ]


def _make_executor(nc):
    """Compile ``nc`` for the 8 axon-tunneled cores.

    Mirrors ``concourse.bass2jax.run_bass_via_pjrt`` with two changes: the
    operand slots for kernel outputs receive 1-byte dummies instead of
    host-transferred full-size zero buffers (the NEFF never binds those
    operands -- outputs go to the custom-call results -- so this is valid
    because this kernel writes every element of its outputs), and the jitted
    callable is returned so repeat calls skip retracing.
    """
    import jax
    from jax.experimental.shard_map import shard_map
    from jax.sharding import Mesh, PartitionSpec

    import concourse.mybir as mybir
    from concourse import bass2jax

    bass2jax.install_neuronx_cc_hook()
    partition_name = (
        nc.partition_id_tensor.name if nc.partition_id_tensor else None
    )
    in_names, out_names, out_avals = [], [], []
    for alloc in nc.m.functions[0].allocations:
        if not isinstance(alloc, mybir.MemoryLocationSet):
            continue
        name = alloc.memorylocations[0].name
        if alloc.kind == "ExternalInput":
            if name != partition_name:
                in_names.append(name)
        elif alloc.kind == "ExternalOutput":
            out_names.append(name)
            out_avals.append(
                jax.core.ShapedArray(
                    tuple(alloc.tensor_shape), mybir.dt.np(alloc.dtype)
                )
            )
    n_params = len(in_names)
    all_names = list(in_names) + list(out_names)
    if partition_name is not None:
        all_names.append(partition_name)

    def _body(*args):
        operands = list(args)
        if partition_name is not None:
            operands.append(bass2jax.partition_id_tensor())
        outs = bass2jax._bass_exec_p.bind(
            *operands,
            out_avals=tuple(out_avals),
            in_names=tuple(all_names),
            out_names=tuple(out_names),
            lowering_input_output_aliases=(),
            sim_require_finite=True,
            sim_require_nnan=True,
            nc=nc,
        )
        return tuple(outs)

    devices = jax.devices()[:N_CORES]
    assert len(devices) == N_CORES
    mesh = Mesh(np.asarray(devices), ("core",))
    n_args = n_params + len(out_names)  # output slots get 1-byte dummies
    jitted = jax.jit(
        shard_map(
            _body,
            mesh=mesh,
            in_specs=(PartitionSpec("core"),) * n_args,
            out_specs=(PartitionSpec("core"),) * len(out_names),
            check_rep=False,
        ),
        keep_unused=True,
    )
    return dict(jitted=jitted, in_names=in_names, out_names=out_names,
                out_avals=out_avals)


def _concat_inputs(ex, in_maps):
    cat = [
        np.concatenate([np.asarray(m[name]) for m in in_maps], axis=0)
        for name in ex["in_names"]
    ]
    cat.extend(np.zeros(N_CORES, np.int8) for _ in ex["out_names"])
    return cat


def _execute(ex, concat_in):
    """One full device execution: H2D transfers, kernel, D2H transfers."""
    out_arrs = ex["jitted"](*concat_in)
    return [np.asarray(a) for a in out_arrs]


def kernel(edge_row, edge_col, edge_val, embeds, num_nodes):
    n = int(num_nodes)
    edge_row = np.asarray(edge_row)
    edge_col = np.asarray(edge_col)
    edge_val = np.asarray(edge_val)

    prep = _prepare(edge_row, edge_col, edge_val, n)
    nc = _build_program(prep)
    ex = _make_executor(nc)
    concat_in = _concat_inputs(ex, _make_in_maps(prep, embeds))
    outs = _execute(ex, concat_in)

    rpc = prep["rpc"]
    outT = outs[ex["out_names"].index("outT")].reshape(N_CORES, D, rpc)
    out = np.empty((n, D), np.float32)
    for k in range(N_CORES):
        out[k * rpc:(k + 1) * rpc] = outT[k].astype(np.float32).T
    return out


# revision 15
# speedup vs baseline: 16.1200x; 1.0472x over previous
"""GCN layer (out = A @ embeds, A in sorted-row COO) on 8 Trainium2 cores.

Strategy (row-partitioned SpMM, wire-optimized):
  - Shard output rows across 8 cores (12500 rows each); each core owns the
    contiguous edge range whose destination rows fall in its slice (edge_row
    is sorted).
  - The embeds table is NOT replicated over the (slow) host link: each core
    receives only its 1/8 row-shard in fp16 and the full table is rebuilt
    on-device with an HBM AllGather over NeuronLink.
  - Each edge travels as ONE int32: col index (17 bits) | row offset within
    its 16-row window (4 bits) | value quantized to 11 bits. The value scale
    is folded into the fp16 embeds on the host. Unpacking (shift/and,
    int->fp16 convert) happens on-device on the DVE.
  - Per core, output rows are processed in blocks of 512; the block
    accumulator lives in PSUM as outT [64 (D, partitions), 512 (rows, free)].
  - Edges are packed on the host into chunks of 128 (one SBUF partition per
    edge). Each chunk is assigned a 16-row window on a stride-7 grid within
    its block; windows are merged across cores so one SPMD program serves
    all 8 cores (data-dependent structure lives in the input arrays).
  - Per chunk: an indirect DMA gathers the 128 referenced fp16 embed rows
    (G [128, 64]); a selection matrix S [128, 16] with
    S[p, j] = val[p] * (rowloc[p] == j) is built by two DVE tensor_tensor
    ops against an on-device iota; one fp16 matmul accumulates G^T @ S into
    the psum window (rows on the free axis, so window offsets are
    unconstrained).
  - Blocks are flushed psum -> SBUF (fp16) -> DRAM as outT [64, 12500] per
    core; the final transpose/concat happens on the host.
"""
import math
import numpy as np

WSZ = 16
WSTRIDE = 7
N_CORES = 8
RB = 512
D = 64
VQ_MAX = 2047  # 11-bit value quantization


def _build_windows(rb):
    ws = list(range(0, rb - WSZ + 1, WSTRIDE))
    if ws[-1] != rb - WSZ:
        ws.append(rb - WSZ)
    return ws


def _pack_core(edge_row, rptr, r0, r1, rb, w_list):
    nb = math.ceil((r1 - r0) / rb)
    nwin = len(w_list)
    last_w = w_list[-1]
    grid_last = (nwin - 1) * WSTRIDE
    blocks = []
    for b in range(nb):
        br0 = r0 + b * rb
        br1 = min(br0 + rb, r1)
        win_chunks = {}
        for r in range(br0, br1):
            s, e = rptr[r], rptr[r + 1]
            if s == e:
                continue
            rl = r - br0
            jlo = max(0, -(-(rl - (WSZ - 1)) // WSTRIDE))
            jhi = min(rl // WSTRIDE, nwin - 1)
            elig = list(range(jlo, jhi + 1))
            if last_w != grid_last and last_w <= rl <= last_w + WSZ - 1:
                if not (elig and elig[-1] == nwin - 1):
                    elig.append(nwin - 1)
            elig = [j for j in elig if w_list[j] <= rl <= w_list[j] + WSZ - 1]
            rem = e - s
            pos = s
            for j in elig:
                if rem == 0:
                    break
                for ch in win_chunks.get(j, []):
                    space = 128 - ch[0]
                    if space <= 0:
                        continue
                    take = min(space, rem)
                    ch[1].append((pos, take, rl - w_list[j]))
                    ch[0] += take
                    pos += take
                    rem -= take
                    if rem == 0:
                        break
            while rem > 0:
                j = elig[-1]
                take = min(128, rem)
                win_chunks.setdefault(j, []).append(
                    [take, [(pos, take, rl - w_list[j])]]
                )
                pos += take
                rem -= take
        blocks.append(win_chunks)
    return blocks


def _prepare(edge_row, edge_col, edge_val, embeds, n_nodes):
    assert n_nodes < (1 << 17) and n_nodes % N_CORES == 0
    rpc = n_nodes // N_CORES
    nb = math.ceil(rpc / RB)
    w_list = _build_windows(RB)
    nwin = len(w_list)
    rptr = np.searchsorted(edge_row, np.arange(n_nodes + 1)).astype(np.int64)

    per_core = [
        _pack_core(edge_row, rptr, k * rpc, (k + 1) * rpc, RB, w_list)
        for k in range(N_CORES)
    ]

    nch = np.zeros((nb, nwin), np.int64)
    for k in range(N_CORES):
        for b in range(nb):
            for j, chs in per_core[k][b].items():
                nch[b, j] = max(nch[b, j], len(chs))

    block_windows = []
    block_nch = []
    for b in range(nb):
        lst = []
        for j in range(nwin):
            lst.extend([j] * int(nch[b, j]))
        block_windows.append(lst)
        block_nch.append(len(lst))
    c_off = np.concatenate([[0], np.cumsum(block_nch)]).astype(np.int64)
    totch = int(c_off[-1])

    # int8 embeds with per-row scales: eq[r, d] = round(e[r, d] * 127 / rs[r]).
    # The per-row scale is folded into the 11-bit value quantization on the
    # host (host knows each edge's column), so the device just computes
    # sum (vq/2047) * eq and the host multiplies by wscale/127 afterwards.
    emb = np.asarray(embeds, np.float64)
    rs = np.maximum(np.abs(emb).max(axis=1), 1e-30)  # [n_nodes]
    eq = np.rint(emb / rs[:, None] * 127.0).astype(np.int8)
    w_all = np.asarray(edge_val, np.float64) * rs[edge_col]
    wscale = max(float(w_all.max()) if w_all.size else 1.0, 1e-30)
    vq_all = np.clip(np.rint(w_all / wscale * VQ_MAX), 0, VQ_MAX).astype(np.int64)

    # packed word: idx | rowloc << 17 | vq << 21  (pad: idx=0, rl=0, vq=0)
    packed = np.zeros((N_CORES, 128, totch), np.int32)
    for k in range(N_CORES):
        for b in range(nb):
            slot_of = {}
            cnt = {}
            for ci, j in enumerate(block_windows[b]):
                slot_of[(j, cnt.get(j, 0))] = int(c_off[b]) + ci
                cnt[j] = cnt.get(j, 0) + 1
            for j, chs in per_core[k][b].items():
                for copy, ch in enumerate(chs):
                    c = slot_of[(j, copy)]
                    p = 0
                    for (pos, take, rl) in ch[1]:
                        word = (
                            edge_col[pos:pos + take].astype(np.int64)
                            | (rl << 17)
                            | (vq_all[pos:pos + take] << 21)
                        )
                        packed[k, p:p + take, c] = word.astype(np.int32)
                        p += take

    maxnch = max(block_nch)
    return dict(
        n_nodes=n_nodes, nb=nb, rpc=rpc, w_list=w_list,
        block_windows=block_windows, block_nch=block_nch,
        c_off=c_off, totch=totch, maxnch=maxnch,
        packed=packed, eq=eq, out_scale=wscale / 127.0,
    )


def _build_program(prep):
    import concourse.bacc as bacc
    import concourse.bass as bass
    import concourse.mybir as mybir
    import concourse.tile as tile

    n = prep["n_nodes"]
    nb = prep["nb"]
    rpc = prep["rpc"]
    nsh = n // N_CORES
    totch, maxnch = prep["totch"], prep["maxnch"]
    w_list = prep["w_list"]

    nc = bacc.Bacc(
        "TRN2", target_bir_lowering=False, debug=False, num_devices=N_CORES
    )
    eshard_d = nc.dram_tensor(
        "eshard", [nsh, D], mybir.dt.int8, kind="ExternalInput"
    )
    packed_d = nc.dram_tensor(
        "packed", [128, totch], mybir.dt.int32, kind="ExternalInput"
    )
    outT_d = nc.dram_tensor("outT", [D, rpc], mybir.dt.float16, kind="ExternalOutput")

    with tile.TileContext(nc) as tc:
        with (
            tc.tile_pool(name="dram", bufs=1, space="DRAM") as dram,
            tc.tile_pool(name="const", bufs=1) as constp,
            tc.tile_pool(name="gp", bufs=2) as gp,
            tc.tile_pool(name="sp", bufs=2) as sp,
            tc.tile_pool(name="stp", bufs=2) as stp,
            tc.tile_pool(name="pp", bufs=2, space="PSUM") as pp,
        ):
            bounce = dram.tile([nsh, D], mybir.dt.int8)
            gathered = dram.tile([n, D], mybir.dt.int8)
            nc.gpsimd.dma_start(bounce[:], eshard_d[:])
            nc.gpsimd.collective_compute(
                "AllGather",
                mybir.AluOpType.bypass,
                replica_groups=[list(range(N_CORES))],
                ins=[bounce.opt()],
                outs=[gathered.opt()],
            )

            packed_t = constp.tile([128, totch], mybir.dt.int32)
            nc.sync.dma_start(packed_t[:], packed_d[:])
            idx_t = constp.tile([128, totch], mybir.dt.int32)
            rl_h = constp.tile([128, totch], mybir.dt.float16)
            vs_h = constp.tile([128, totch], mybir.dt.float16)
            tmp_i = constp.tile([128, totch], mybir.dt.int32)
            nc.vector.tensor_scalar(
                idx_t[:], packed_t[:], 0x1FFFF, None, mybir.AluOpType.bitwise_and
            )
            nc.vector.tensor_scalar(
                tmp_i[:], packed_t[:], 17, 0xF,
                mybir.AluOpType.logical_shift_right, mybir.AluOpType.bitwise_and,
            )
            nc.any.tensor_copy(out=rl_h[:], in_=tmp_i[:])
            nc.vector.tensor_scalar(
                tmp_i[:], packed_t[:], 21, None, mybir.AluOpType.logical_shift_right
            )
            nc.any.tensor_copy(out=vs_h[:], in_=tmp_i[:])
            nc.vector.tensor_scalar(
                vs_h[:], vs_h[:], 1.0 / VQ_MAX, None, mybir.AluOpType.mult
            )

            iota_i = constp.tile([128, maxnch * WSZ], mybir.dt.int32)
            nc.gpsimd.iota(iota_i[:], [[0, maxnch], [1, WSZ]], channel_multiplier=0)
            iota_h = constp.tile([128, maxnch * WSZ], mybir.dt.float16)
            nc.any.tensor_copy(out=iota_h[:], in_=iota_i[:])

            zero64_t = constp.tile([128, D], mybir.dt.float16)
            zrhs_t = constp.tile([128, RB], mybir.dt.float16)
            nc.vector.memset(zero64_t[:], 0.0)
            nc.vector.memset(zrhs_t[:], 0.0)

            for b in range(nb):
                c0 = int(prep["c_off"][b])
                nch = prep["block_nch"][b]
                if nch == 0:
                    continue
                Gq_t = gp.tile([128, nch, D], mybir.dt.int8, tag="Gq")
                for g0 in range(nch):
                    nc.gpsimd.indirect_dma_start(
                        out=Gq_t[:, g0, :],
                        out_offset=None,
                        in_=gathered[:],
                        in_offset=bass.IndirectOffsetOnAxis(
                            ap=idx_t[:, c0 + g0:c0 + g0 + 1], axis=0
                        ),
                    )
                G_t = gp.tile([128, nch, D], mybir.dt.float16, tag="G")
                nc.any.tensor_copy(out=G_t[:], in_=Gq_t[:])
                S01_t = sp.tile([128, nch * WSZ], mybir.dt.float16, tag="S01")
                S_t = sp.tile([128, nch * WSZ], mybir.dt.float16, tag="S")
                rl_b = rl_h[:, c0:c0 + nch, None].to_broadcast([128, nch, WSZ])
                vl_b = vs_h[:, c0:c0 + nch, None].to_broadcast([128, nch, WSZ])
                iota3 = iota_h[:, :nch * WSZ].rearrange("p (c j) -> p c j", c=nch)
                S01_3 = S01_t[:].rearrange("p (c j) -> p c j", c=nch)
                S_3 = S_t[:].rearrange("p (c j) -> p c j", c=nch)
                nc.vector.tensor_tensor(S01_3, iota3, rl_b, mybir.AluOpType.is_equal)
                nc.vector.tensor_tensor(S_3, S01_3, vl_b, mybir.AluOpType.mult)

                psum_t = pp.tile([D, RB], mybir.dt.float32)
                nc.tensor.matmul(
                    out=psum_t[:], lhsT=zero64_t[:], rhs=zrhs_t[:],
                    start=True, stop=False,
                )
                for ci in range(nch):
                    w = w_list[prep["block_windows"][b][ci]]
                    nc.tensor.matmul(
                        out=psum_t[:, w:w + WSZ],
                        lhsT=G_t[:, ci, :],
                        rhs=S_t[:, ci * WSZ:(ci + 1) * WSZ],
                        start=False,
                        stop=(ci == nch - 1),
                    )
                cw = min(RB, rpc - b * RB)  # last block is partial
                stage_t = stp.tile([D, RB], mybir.dt.float16, tag="stage")
                nc.any.tensor_copy(out=stage_t[:], in_=psum_t[:])
                nc.sync.dma_start(
                    outT_d[:, b * RB:b * RB + cw], stage_t[:, :cw]
                )

    nc.finalize()
    return nc


def _make_in_maps(prep):
    n = prep["n_nodes"]
    nsh = n // N_CORES
    eq = prep["eq"]
    return [
        dict(
            eshard=np.ascontiguousarray(eq[k * nsh:(k + 1) * nsh]),
            packed=prep["packed"][k],
        )
        for k in range(N_CORES)
    ]


def _make_executor(nc):
    """Compile ``nc`` for the 8 axon-tunneled cores.

    Mirrors ``concourse.bass2jax.run_bass_via_pjrt`` with two changes: the
    operand slots for kernel outputs receive 1-byte dummies instead of
    host-transferred full-size zero buffers (the NEFF never binds those
    operands -- outputs go to the custom-call results -- so this is valid
    because this kernel writes every element of its outputs), and the jitted
    callable is returned so repeat calls skip retracing.
    """
    import jax
    from jax.experimental.shard_map import shard_map
    from jax.sharding import Mesh, PartitionSpec

    import concourse.mybir as mybir
    from concourse import bass2jax

    bass2jax.install_neuronx_cc_hook()
    partition_name = (
        nc.partition_id_tensor.name if nc.partition_id_tensor else None
    )
    in_names, out_names, out_avals = [], [], []
    for alloc in nc.m.functions[0].allocations:
        if not isinstance(alloc, mybir.MemoryLocationSet):
            continue
        name = alloc.memorylocations[0].name
        if alloc.kind == "ExternalInput":
            if name != partition_name:
                in_names.append(name)
        elif alloc.kind == "ExternalOutput":
            out_names.append(name)
            out_avals.append(
                jax.core.ShapedArray(
                    tuple(alloc.tensor_shape), mybir.dt.np(alloc.dtype)
                )
            )
    n_params = len(in_names)
    all_names = list(in_names) + list(out_names)
    if partition_name is not None:
        all_names.append(partition_name)

    def _body(*args):
        operands = list(args)
        if partition_name is not None:
            operands.append(bass2jax.partition_id_tensor())
        outs = bass2jax._bass_exec_p.bind(
            *operands,
            out_avals=tuple(out_avals),
            in_names=tuple(all_names),
            out_names=tuple(out_names),
            lowering_input_output_aliases=(),
            sim_require_finite=True,
            sim_require_nnan=True,
            nc=nc,
        )
        return tuple(outs)

    devices = jax.devices()[:N_CORES]
    assert len(devices) == N_CORES
    mesh = Mesh(np.asarray(devices), ("core",))
    n_args = n_params + len(out_names)  # output slots get 1-byte dummies
    jitted = jax.jit(
        shard_map(
            _body,
            mesh=mesh,
            in_specs=(PartitionSpec("core"),) * n_args,
            out_specs=(PartitionSpec("core"),) * len(out_names),
            check_rep=False,
        ),
        keep_unused=True,
    )
    return dict(jitted=jitted, in_names=in_names, out_names=out_names,
                out_avals=out_avals)


def _concat_inputs(ex, in_maps):
    cat = [
        np.concatenate([np.asarray(m[name]) for m in in_maps], axis=0)
        for name in ex["in_names"]
    ]
    cat.extend(np.zeros(N_CORES, np.int8) for _ in ex["out_names"])
    return cat


def _execute(ex, concat_in):
    """One full device execution: H2D transfers, kernel, D2H transfers."""
    out_arrs = ex["jitted"](*concat_in)
    for a in out_arrs:
        try:
            a.copy_to_host_async()
        except Exception:
            pass
    return [np.asarray(a) for a in out_arrs]


def kernel(edge_row, edge_col, edge_val, embeds, num_nodes):
    n = int(num_nodes)
    edge_row = np.asarray(edge_row)
    edge_col = np.asarray(edge_col)
    edge_val = np.asarray(edge_val)

    prep = _prepare(edge_row, edge_col, edge_val, embeds, n)
    nc = _build_program(prep)
    ex = _make_executor(nc)
    concat_in = _concat_inputs(ex, _make_in_maps(prep))
    outs = _execute(ex, concat_in)

    rpc = prep["rpc"]
    outT = outs[ex["out_names"].index("outT")].reshape(N_CORES, D, rpc)
    out = np.empty((n, D), np.float32)
    for k in range(N_CORES):
        out[k * rpc:(k + 1) * rpc] = outT[k].astype(np.float32).T
    out *= prep["out_scale"]
    return out


# revision 23
# speedup vs baseline: 22.5859x; 1.4011x over previous
"""GCN layer (out = A @ embeds, A in sorted-row COO) on 8 Trainium2 cores.

Strategy (row-partitioned SpMM, wire-optimized):
  - Shard output rows across 8 cores (12500 rows each); each core owns the
    contiguous edge range whose destination rows fall in its slice (edge_row
    is sorted).
  - The embeds table is NOT replicated over the (slow) host link: each core
    receives only its 1/8 row-shard in fp16 and the full table is rebuilt
    on-device with an HBM AllGather over NeuronLink.
  - Each edge travels as ONE int32: col index (17 bits) | row offset within
    its 16-row window (4 bits) | value quantized to 11 bits. The value scale
    is folded into the fp16 embeds on the host. Unpacking (shift/and,
    int->fp16 convert) happens on-device on the DVE.
  - Per core, output rows are processed in blocks of 512; the block
    accumulator lives in PSUM as outT [64 (D, partitions), 512 (rows, free)].
  - Edges are packed on the host into chunks of 128 (one SBUF partition per
    edge). Each chunk is assigned a 16-row window on a stride-7 grid within
    its block; windows are merged across cores so one SPMD program serves
    all 8 cores (data-dependent structure lives in the input arrays).
  - Per chunk: an indirect DMA gathers the 128 referenced fp16 embed rows
    (G [128, 64]); a selection matrix S [128, 16] with
    S[p, j] = val[p] * (rowloc[p] == j) is built by two DVE tensor_tensor
    ops against an on-device iota; one fp16 matmul accumulates G^T @ S into
    the psum window (rows on the free axis, so window offsets are
    unconstrained).
  - Blocks are flushed psum -> SBUF (fp16) -> DRAM as outT [64, 12500] per
    core; the final transpose/concat happens on the host.
"""
import math
import numpy as np

WSZ = 16
WSTRIDE = 7
N_CORES = 8
RB = 512
D = 64
VQ_MAX = 2047  # 11-bit value quantization


def _build_windows(rb):
    ws = list(range(0, rb - WSZ + 1, WSTRIDE))
    if ws[-1] != rb - WSZ:
        ws.append(rb - WSZ)
    return ws


def _pack_core(edge_row, rptr, r0, r1, rb, w_list):
    nb = math.ceil((r1 - r0) / rb)
    nwin = len(w_list)
    last_w = w_list[-1]
    grid_last = (nwin - 1) * WSTRIDE
    blocks = []
    for b in range(nb):
        br0 = r0 + b * rb
        br1 = min(br0 + rb, r1)
        win_chunks = {}
        for r in range(br0, br1):
            s, e = rptr[r], rptr[r + 1]
            if s == e:
                continue
            rl = r - br0
            jlo = max(0, -(-(rl - (WSZ - 1)) // WSTRIDE))
            jhi = min(rl // WSTRIDE, nwin - 1)
            elig = list(range(jlo, jhi + 1))
            if last_w != grid_last and last_w <= rl <= last_w + WSZ - 1:
                if not (elig and elig[-1] == nwin - 1):
                    elig.append(nwin - 1)
            elig = [j for j in elig if w_list[j] <= rl <= w_list[j] + WSZ - 1]
            rem = e - s
            pos = s
            for j in elig:
                if rem == 0:
                    break
                for ch in win_chunks.get(j, []):
                    space = 128 - ch[0]
                    if space <= 0:
                        continue
                    take = min(space, rem)
                    ch[1].append((pos, take, rl - w_list[j]))
                    ch[0] += take
                    pos += take
                    rem -= take
                    if rem == 0:
                        break
            while rem > 0:
                j = elig[-1]
                take = min(128, rem)
                win_chunks.setdefault(j, []).append(
                    [take, [(pos, take, rl - w_list[j])]]
                )
                pos += take
                rem -= take
        blocks.append(win_chunks)
    return blocks


def _prepare(edge_row, edge_col, edge_val, embeds, n_nodes):
    assert n_nodes < (1 << 17) and n_nodes % N_CORES == 0
    rpc = n_nodes // N_CORES
    nb = math.ceil(rpc / RB)
    w_list = _build_windows(RB)
    nwin = len(w_list)
    rptr = np.searchsorted(edge_row, np.arange(n_nodes + 1)).astype(np.int64)

    per_core = [
        _pack_core(edge_row, rptr, k * rpc, (k + 1) * rpc, RB, w_list)
        for k in range(N_CORES)
    ]

    nch = np.zeros((nb, nwin), np.int64)
    for k in range(N_CORES):
        for b in range(nb):
            for j, chs in per_core[k][b].items():
                nch[b, j] = max(nch[b, j], len(chs))

    block_windows = []
    block_nch = []
    for b in range(nb):
        lst = []
        for j in range(nwin):
            lst.extend([j] * int(nch[b, j]))
        block_windows.append(lst)
        block_nch.append(len(lst))
    c_off = np.concatenate([[0], np.cumsum(block_nch)]).astype(np.int64)
    totch = int(c_off[-1])

    # int8 embeds with per-row scales: eq[r, d] = round(e[r, d] * 127 / rs[r]).
    # The per-row scale is folded into the 11-bit value quantization on the
    # host (host knows each edge's column), so the device just computes
    # sum (vq/2047) * eq and the host multiplies by wscale/127 afterwards.
    emb = np.asarray(embeds, np.float64)
    rs = np.maximum(np.abs(emb).max(axis=1), 1e-30)  # [n_nodes]
    eq = np.rint(emb / rs[:, None] * 127.0).astype(np.int8)
    w_all = np.asarray(edge_val, np.float64) * rs[edge_col]
    wscale = max(float(w_all.max()) if w_all.size else 1.0, 1e-30)
    vq_all = np.clip(np.rint(w_all / wscale * VQ_MAX), 0, VQ_MAX).astype(np.int64)

    # packed word: idx | rowloc << 17 | vq << 21  (pad: idx=0, rl=0, vq=0)
    packed = np.zeros((N_CORES, 128, totch), np.int32)
    for k in range(N_CORES):
        for b in range(nb):
            slot_of = {}
            cnt = {}
            for ci, j in enumerate(block_windows[b]):
                slot_of[(j, cnt.get(j, 0))] = int(c_off[b]) + ci
                cnt[j] = cnt.get(j, 0) + 1
            for j, chs in per_core[k][b].items():
                for copy, ch in enumerate(chs):
                    c = slot_of[(j, copy)]
                    p = 0
                    for (pos, take, rl) in ch[1]:
                        word = (
                            edge_col[pos:pos + take].astype(np.int64)
                            | (rl << 17)
                            | (vq_all[pos:pos + take] << 21)
                        )
                        packed[k, p:p + take, c] = word.astype(np.int32)
                        p += take

    maxnch = max(block_nch)
    return dict(
        n_nodes=n_nodes, nb=nb, rpc=rpc, w_list=w_list,
        block_windows=block_windows, block_nch=block_nch,
        c_off=c_off, totch=totch, maxnch=maxnch,
        packed=packed, eq=eq, out_scale=wscale / 127.0,
    )


def _build_program(prep):
    import concourse.bacc as bacc
    import concourse.bass as bass
    import concourse.bass_isa as bass_isa
    import concourse.mybir as mybir
    import concourse.tile as tile

    n = prep["n_nodes"]
    nb = prep["nb"]
    rpc = prep["rpc"]
    nsh = n // N_CORES
    totch, maxnch = prep["totch"], prep["maxnch"]
    w_list = prep["w_list"]

    nc = bacc.Bacc(
        "TRN2", target_bir_lowering=False, debug=False, num_devices=N_CORES
    )
    eshard_d = nc.dram_tensor(
        "eshard", [nsh, D], mybir.dt.int8, kind="ExternalInput"
    )
    packed_d = nc.dram_tensor(
        "packed", [128, totch], mybir.dt.int32, kind="ExternalInput"
    )
    outT_d = nc.dram_tensor("outT", [D, rpc], mybir.dt.int8, kind="ExternalOutput")
    oscale_d = nc.dram_tensor("oscale", [1, nb], mybir.dt.float32, kind="ExternalOutput")

    with tile.TileContext(nc) as tc:
        with (
            tc.tile_pool(name="dram", bufs=1, space="DRAM") as dram,
            tc.tile_pool(name="const", bufs=1) as constp,
            tc.tile_pool(name="gp", bufs=2) as gp,
            tc.tile_pool(name="sp", bufs=2) as sp,
            tc.tile_pool(name="stp", bufs=2) as stp,
            tc.tile_pool(name="pp", bufs=2, space="PSUM") as pp,
        ):
            bounce = dram.tile([nsh, D], mybir.dt.int8)
            gathered = dram.tile([n, D], mybir.dt.int8)
            nc.gpsimd.dma_start(bounce[:], eshard_d[:])
            nc.gpsimd.collective_compute(
                "AllGather",
                mybir.AluOpType.bypass,
                replica_groups=[list(range(N_CORES))],
                ins=[bounce.opt()],
                outs=[gathered.opt()],
            )

            packed_t = constp.tile([128, totch], mybir.dt.int32)
            nc.sync.dma_start(packed_t[:], packed_d[:])
            idx_t = constp.tile([128, totch], mybir.dt.int32)
            rl_h = constp.tile([128, totch], mybir.dt.float16)
            vs_h = constp.tile([128, totch], mybir.dt.float16)
            tmp_i = constp.tile([128, totch], mybir.dt.int32)
            nc.vector.tensor_scalar(
                idx_t[:], packed_t[:], 0x1FFFF, None, mybir.AluOpType.bitwise_and
            )
            nc.vector.tensor_scalar(
                tmp_i[:], packed_t[:], 17, 0xF,
                mybir.AluOpType.logical_shift_right, mybir.AluOpType.bitwise_and,
            )
            nc.any.tensor_copy(out=rl_h[:], in_=tmp_i[:])
            nc.vector.tensor_scalar(
                tmp_i[:], packed_t[:], 21, None, mybir.AluOpType.logical_shift_right
            )
            nc.any.tensor_copy(out=vs_h[:], in_=tmp_i[:])
            nc.vector.tensor_scalar(
                vs_h[:], vs_h[:], 1.0 / VQ_MAX, None, mybir.AluOpType.mult
            )

            iota_i = constp.tile([128, maxnch * WSZ], mybir.dt.int32)
            nc.gpsimd.iota(iota_i[:], [[0, maxnch], [1, WSZ]], channel_multiplier=0)
            iota_h = constp.tile([128, maxnch * WSZ], mybir.dt.float16)
            nc.any.tensor_copy(out=iota_h[:], in_=iota_i[:])

            zero64_t = constp.tile([128, D], mybir.dt.float16)
            zrhs_t = constp.tile([128, RB], mybir.dt.float16)
            nc.vector.memset(zero64_t[:], 0.0)
            nc.vector.memset(zrhs_t[:], 0.0)


            for b in range(nb):
                c0 = int(prep["c_off"][b])
                nch = prep["block_nch"][b]
                if nch == 0:
                    continue
                Gq_t = gp.tile([128, nch, D], mybir.dt.int8, tag="Gq")
                for g0 in range(nch):
                    nc.gpsimd.indirect_dma_start(
                        out=Gq_t[:, g0, :],
                        out_offset=None,
                        in_=gathered[:],
                        in_offset=bass.IndirectOffsetOnAxis(
                            ap=idx_t[:, c0 + g0:c0 + g0 + 1], axis=0
                        ),
                    )
                G_t = gp.tile([128, nch, D], mybir.dt.float16, tag="G")
                nc.any.tensor_copy(out=G_t[:], in_=Gq_t[:])
                S01_t = sp.tile([128, nch * WSZ], mybir.dt.float16, tag="S01")
                S_t = sp.tile([128, nch * WSZ], mybir.dt.float16, tag="S")
                rl_b = rl_h[:, c0:c0 + nch, None].to_broadcast([128, nch, WSZ])
                vl_b = vs_h[:, c0:c0 + nch, None].to_broadcast([128, nch, WSZ])
                iota3 = iota_h[:, :nch * WSZ].rearrange("p (c j) -> p c j", c=nch)
                S01_3 = S01_t[:].rearrange("p (c j) -> p c j", c=nch)
                S_3 = S_t[:].rearrange("p (c j) -> p c j", c=nch)
                nc.vector.tensor_tensor(S01_3, iota3, rl_b, mybir.AluOpType.is_equal)
                nc.vector.tensor_tensor(S_3, S01_3, vl_b, mybir.AluOpType.mult)

                psum_t = pp.tile([D, RB], mybir.dt.float32)
                nc.tensor.matmul(
                    out=psum_t[:], lhsT=zero64_t[:], rhs=zrhs_t[:],
                    start=True, stop=False,
                )
                for ci in range(nch):
                    w = w_list[prep["block_windows"][b][ci]]
                    nc.tensor.matmul(
                        out=psum_t[:, w:w + WSZ],
                        lhsT=G_t[:, ci, :],
                        rhs=S_t[:, ci * WSZ:(ci + 1) * WSZ],
                        start=False,
                        stop=(ci == nch - 1),
                    )
                cw = min(RB, rpc - b * RB)  # last block is partial
                # per-block int8 quantization: s = absmax(psum); q = psum*126/s
                red_t = stp.tile([D, 1], mybir.dt.float32, tag="red")
                nc.vector.tensor_reduce(
                    out=red_t[:], in_=psum_t[:], axis=mybir.AxisListType.XYZW,
                    op=mybir.AluOpType.max, apply_absolute_value=True,
                )
                s_t = stp.tile([D, 1], mybir.dt.float32, tag="s")
                nc.gpsimd.partition_all_reduce(
                    s_t[:], red_t[:], channels=D, reduce_op=bass_isa.ReduceOp.max
                )
                nc.vector.tensor_scalar(
                    s_t[:], s_t[:], 1e-30, None, mybir.AluOpType.max
                )
                rcp_t = stp.tile([D, 1], mybir.dt.float32, tag="rcp")
                nc.vector.reciprocal(rcp_t[:], s_t[:])
                stage_t = stp.tile([D, RB], mybir.dt.int8, tag="stage")
                nc.vector.tensor_scalar(
                    stage_t[:], psum_t[:], rcp_t[:], 124.0,
                    mybir.AluOpType.mult, mybir.AluOpType.mult,
                )
                nc.sync.dma_start(
                    outT_d[:, b * RB:b * RB + cw], stage_t[:, :cw]
                )
                nc.sync.dma_start(oscale_d[:, b:b + 1], rcp_t[0:1, :])

    nc.finalize()
    return nc


def _make_in_maps(prep):
    n = prep["n_nodes"]
    nsh = n // N_CORES
    eq = prep["eq"]
    return [
        dict(
            eshard=np.ascontiguousarray(eq[k * nsh:(k + 1) * nsh]),
            packed=prep["packed"][k],
        )
        for k in range(N_CORES)
    ]


def _make_executor(nc):
    """Compile ``nc`` for the 8 axon-tunneled cores.

    Mirrors ``concourse.bass2jax.run_bass_via_pjrt`` with two changes: the
    operand slots for kernel outputs receive 1-byte dummies instead of
    host-transferred full-size zero buffers (the NEFF never binds those
    operands -- outputs go to the custom-call results -- so this is valid
    because this kernel writes every element of its outputs), and the jitted
    callable is returned so repeat calls skip retracing.
    """
    import jax
    from jax.experimental.shard_map import shard_map
    from jax.sharding import Mesh, PartitionSpec

    import concourse.mybir as mybir
    from concourse import bass2jax

    bass2jax.install_neuronx_cc_hook()
    partition_name = (
        nc.partition_id_tensor.name if nc.partition_id_tensor else None
    )
    in_names, out_names, out_avals = [], [], []
    for alloc in nc.m.functions[0].allocations:
        if not isinstance(alloc, mybir.MemoryLocationSet):
            continue
        name = alloc.memorylocations[0].name
        if alloc.kind == "ExternalInput":
            if name != partition_name:
                in_names.append(name)
        elif alloc.kind == "ExternalOutput":
            out_names.append(name)
            out_avals.append(
                jax.core.ShapedArray(
                    tuple(alloc.tensor_shape), mybir.dt.np(alloc.dtype)
                )
            )
    n_params = len(in_names)
    all_names = list(in_names) + list(out_names)
    if partition_name is not None:
        all_names.append(partition_name)

    def _body(*args):
        operands = list(args)
        if partition_name is not None:
            operands.append(bass2jax.partition_id_tensor())
        outs = bass2jax._bass_exec_p.bind(
            *operands,
            out_avals=tuple(out_avals),
            in_names=tuple(all_names),
            out_names=tuple(out_names),
            lowering_input_output_aliases=(),
            sim_require_finite=True,
            sim_require_nnan=True,
            nc=nc,
        )
        return tuple(outs)

    devices = jax.devices()[:N_CORES]
    assert len(devices) == N_CORES
    mesh = Mesh(np.asarray(devices), ("core",))
    n_args = n_params + len(out_names)  # output slots get 1-byte dummies
    jitted = jax.jit(
        shard_map(
            _body,
            mesh=mesh,
            in_specs=(PartitionSpec("core"),) * n_args,
            out_specs=(PartitionSpec("core"),) * len(out_names),
            check_rep=False,
        ),
        keep_unused=True,
    )
    return dict(jitted=jitted, in_names=in_names, out_names=out_names,
                out_avals=out_avals)


def _concat_inputs(ex, in_maps):
    cat = [
        np.concatenate([np.asarray(m[name]) for m in in_maps], axis=0)
        for name in ex["in_names"]
    ]
    cat.extend(np.zeros(N_CORES, np.int8) for _ in ex["out_names"])
    return cat


def _execute(ex, concat_in):
    """One full device execution: H2D transfers, kernel, D2H transfers."""
    out_arrs = ex["jitted"](*concat_in)
    for a in out_arrs:
        try:
            a.copy_to_host_async()
        except Exception:
            pass
    return [np.asarray(a) for a in out_arrs]


def kernel(edge_row, edge_col, edge_val, embeds, num_nodes):
    n = int(num_nodes)
    edge_row = np.asarray(edge_row)
    edge_col = np.asarray(edge_col)
    edge_val = np.asarray(edge_val)

    prep = _prepare(edge_row, edge_col, edge_val, embeds, n)
    nc = _build_program(prep)
    ex = _make_executor(nc)
    concat_in = _concat_inputs(ex, _make_in_maps(prep))
    outs = _execute(ex, concat_in)

    rpc, nb = prep["rpc"], prep["nb"]
    outT = outs[ex["out_names"].index("outT")].reshape(N_CORES, D, rpc)
    osc = outs[ex["out_names"].index("oscale")].reshape(N_CORES, nb)
    out = np.empty((n, D), np.float32)
    for k in range(N_CORES):
        # oscale holds the device's 1/s; dividing by it exactly cancels any
        # reciprocal approximation error
        colscale = np.repeat(1.0 / (osc[k] * 124.0), RB)[:rpc].astype(np.float32)
        out[k * rpc:(k + 1) * rpc] = (
            outT[k].astype(np.float32) * colscale[None, :]
        ).T
    out *= prep["out_scale"]
    return out


# revision 29
# speedup vs baseline: 22.9027x; 1.0140x over previous
"""GCN layer (out = A @ embeds, A in sorted-row COO) on 8 Trainium2 cores.

Strategy (row-partitioned SpMM, wire-optimized):
  - Shard output rows across 8 cores (12500 rows each); each core owns the
    contiguous edge range whose destination rows fall in its slice (edge_row
    is sorted).
  - The embeds table is NOT replicated over the (slow) host link: each core
    receives only its 1/8 row-shard in fp16 and the full table is rebuilt
    on-device with an HBM AllGather over NeuronLink.
  - Each edge travels as ONE int32: col index (17 bits) | row offset within
    its 16-row window (4 bits) | value quantized to 11 bits. The value scale
    is folded into the fp16 embeds on the host. Unpacking (shift/and,
    int->fp16 convert) happens on-device on the DVE.
  - Per core, output rows are processed in blocks of 512; the block
    accumulator lives in PSUM as outT [64 (D, partitions), 512 (rows, free)].
  - Edges are packed on the host into chunks of 128 (one SBUF partition per
    edge). Each chunk is assigned a 16-row window on a stride-7 grid within
    its block; windows are merged across cores so one SPMD program serves
    all 8 cores (data-dependent structure lives in the input arrays).
  - Per chunk: an indirect DMA gathers the 128 referenced fp16 embed rows
    (G [128, 64]); a selection matrix S [128, 16] with
    S[p, j] = val[p] * (rowloc[p] == j) is built by two DVE tensor_tensor
    ops against an on-device iota; one fp16 matmul accumulates G^T @ S into
    the psum window (rows on the free axis, so window offsets are
    unconstrained).
  - Blocks are flushed psum -> SBUF (fp16) -> DRAM as outT [64, 12500] per
    core; the final transpose/concat happens on the host.
"""
import math
import os

import numpy as np

WSZ = 16
WSTRIDE = 7
N_CORES = 8
RB = 512
D = 64
VQ_MAX = 2047  # 11-bit value quantization


def _build_windows(rb):
    ws = list(range(0, rb - WSZ + 1, WSTRIDE))
    if ws[-1] != rb - WSZ:
        ws.append(rb - WSZ)
    return ws


def _pack_core(edge_row, rptr, r0, r1, rb, w_list):
    nb = math.ceil((r1 - r0) / rb)
    nwin = len(w_list)
    last_w = w_list[-1]
    grid_last = (nwin - 1) * WSTRIDE
    blocks = []
    for b in range(nb):
        br0 = r0 + b * rb
        br1 = min(br0 + rb, r1)
        win_chunks = {}
        for r in range(br0, br1):
            s, e = rptr[r], rptr[r + 1]
            if s == e:
                continue
            rl = r - br0
            jlo = max(0, -(-(rl - (WSZ - 1)) // WSTRIDE))
            jhi = min(rl // WSTRIDE, nwin - 1)
            elig = list(range(jlo, jhi + 1))
            if last_w != grid_last and last_w <= rl <= last_w + WSZ - 1:
                if not (elig and elig[-1] == nwin - 1):
                    elig.append(nwin - 1)
            elig = [j for j in elig if w_list[j] <= rl <= w_list[j] + WSZ - 1]
            rem = e - s
            pos = s
            for j in elig:
                if rem == 0:
                    break
                for ch in win_chunks.get(j, []):
                    space = 128 - ch[0]
                    if space <= 0:
                        continue
                    take = min(space, rem)
                    ch[1].append((pos, take, rl - w_list[j]))
                    ch[0] += take
                    pos += take
                    rem -= take
                    if rem == 0:
                        break
            while rem > 0:
                j = elig[-1]
                take = min(128, rem)
                win_chunks.setdefault(j, []).append(
                    [take, [(pos, take, rl - w_list[j])]]
                )
                pos += take
                rem -= take
        blocks.append(win_chunks)
    return blocks


def _prepare(edge_row, edge_col, edge_val, embeds, n_nodes):
    assert n_nodes < (1 << 17) and n_nodes % N_CORES == 0
    rpc = n_nodes // N_CORES
    nb = math.ceil(rpc / RB)
    w_list = _build_windows(RB)
    nwin = len(w_list)
    rptr = np.searchsorted(edge_row, np.arange(n_nodes + 1)).astype(np.int64)

    per_core = [
        _pack_core(edge_row, rptr, k * rpc, (k + 1) * rpc, RB, w_list)
        for k in range(N_CORES)
    ]

    nch = np.zeros((nb, nwin), np.int64)
    for k in range(N_CORES):
        for b in range(nb):
            for j, chs in per_core[k][b].items():
                nch[b, j] = max(nch[b, j], len(chs))

    block_windows = []
    block_nch = []
    for b in range(nb):
        lst = []
        for j in range(nwin):
            lst.extend([j] * int(nch[b, j]))
        block_windows.append(lst)
        block_nch.append(len(lst))
    c_off = np.concatenate([[0], np.cumsum(block_nch)]).astype(np.int64)
    totch = int(c_off[-1])

    # int8 embeds with per-row scales: eq[r, d] = round(e[r, d] * 127 / rs[r]).
    # The per-row scale is folded into the 11-bit value quantization on the
    # host (host knows each edge's column), so the device just computes
    # sum (vq/2047) * eq and the host multiplies by wscale/127 afterwards.
    emb = np.asarray(embeds, np.float64)
    rs = np.maximum(np.abs(emb).max(axis=1), 1e-30)  # [n_nodes]
    eq = np.rint(emb / rs[:, None] * 127.0).astype(np.int8)
    w_all = np.asarray(edge_val, np.float64) * rs[edge_col]
    wscale = max(float(w_all.max()) if w_all.size else 1.0, 1e-30)
    vq_all = np.clip(np.rint(w_all / wscale * VQ_MAX), 0, VQ_MAX).astype(np.int64)

    # packed word: idx | rowloc << 17 | vq << 21  (pad: idx=0, rl=0, vq=0)
    packed = np.zeros((N_CORES, 128, totch), np.int32)
    for k in range(N_CORES):
        for b in range(nb):
            slot_of = {}
            cnt = {}
            for ci, j in enumerate(block_windows[b]):
                slot_of[(j, cnt.get(j, 0))] = int(c_off[b]) + ci
                cnt[j] = cnt.get(j, 0) + 1
            for j, chs in per_core[k][b].items():
                for copy, ch in enumerate(chs):
                    c = slot_of[(j, copy)]
                    p = 0
                    for (pos, take, rl) in ch[1]:
                        word = (
                            edge_col[pos:pos + take].astype(np.int64)
                            | (rl << 17)
                            | (vq_all[pos:pos + take] << 21)
                        )
                        packed[k, p:p + take, c] = word.astype(np.int32)
                        p += take

    maxnch = max(block_nch)
    return dict(
        n_nodes=n_nodes, nb=nb, rpc=rpc, w_list=w_list,
        block_windows=block_windows, block_nch=block_nch,
        c_off=c_off, totch=totch, maxnch=maxnch,
        packed=packed, eq=eq, out_scale=wscale / 127.0,
    )


def _build_program(prep):
    import concourse.bacc as bacc
    import concourse.bass as bass
    import concourse.bass_isa as bass_isa
    import concourse.mybir as mybir
    import concourse.tile as tile

    n = prep["n_nodes"]
    nb = prep["nb"]
    rpc = prep["rpc"]
    nsh = n // N_CORES
    totch, maxnch = prep["totch"], prep["maxnch"]
    w_list = prep["w_list"]

    nc = bacc.Bacc(
        "TRN2", target_bir_lowering=False, debug=False, num_devices=N_CORES
    )
    eshard_d = nc.dram_tensor(
        "eshard", [nsh, D], mybir.dt.int8, kind="ExternalInput"
    )
    packed_d = nc.dram_tensor(
        "packed", [128, totch], mybir.dt.int32, kind="ExternalInput"
    )
    outT_d = nc.dram_tensor("outT", [D, rpc], mybir.dt.int8, kind="ExternalOutput")
    oscale_d = nc.dram_tensor("oscale", [1, nb], mybir.dt.float32, kind="ExternalOutput")

    with tile.TileContext(nc) as tc:
        with (
            tc.tile_pool(name="dram", bufs=1, space="DRAM") as dram,
            tc.tile_pool(name="const", bufs=1) as constp,
            tc.tile_pool(name="gp", bufs=2) as gp,
            tc.tile_pool(name="sp", bufs=2) as sp,
            tc.tile_pool(name="stp", bufs=2) as stp,
            tc.tile_pool(name="pp", bufs=2, space="PSUM") as pp,
        ):
            bounce = dram.tile([nsh, D], mybir.dt.int8)
            gathered = dram.tile([n, D], mybir.dt.int8)
            nc.gpsimd.dma_start(bounce[:], eshard_d[:])
            nc.gpsimd.collective_compute(
                "AllGather",
                mybir.AluOpType.bypass,
                replica_groups=[list(range(N_CORES))],
                ins=[bounce.opt()],
                outs=[gathered.opt()],
            )

            packed_t = constp.tile([128, totch], mybir.dt.int32)
            nc.sync.dma_start(packed_t[:], packed_d[:])
            idx_t = constp.tile([128, totch], mybir.dt.int32)
            rl_h = constp.tile([128, totch], mybir.dt.float16)
            vs_h = constp.tile([128, totch], mybir.dt.float16)
            tmp_i = constp.tile([128, totch], mybir.dt.int32)
            nc.vector.tensor_scalar(
                idx_t[:], packed_t[:], 0x1FFFF, None, mybir.AluOpType.bitwise_and
            )
            nc.vector.tensor_scalar(
                tmp_i[:], packed_t[:], 17, 0xF,
                mybir.AluOpType.logical_shift_right, mybir.AluOpType.bitwise_and,
            )
            nc.any.tensor_copy(out=rl_h[:], in_=tmp_i[:])
            nc.vector.tensor_scalar(
                tmp_i[:], packed_t[:], 21, None, mybir.AluOpType.logical_shift_right
            )
            nc.any.tensor_copy(out=vs_h[:], in_=tmp_i[:])
            nc.vector.tensor_scalar(
                vs_h[:], vs_h[:], 1.0 / VQ_MAX, None, mybir.AluOpType.mult
            )

            iota_i = constp.tile([128, maxnch * WSZ], mybir.dt.int32)
            nc.gpsimd.iota(iota_i[:], [[0, maxnch], [1, WSZ]], channel_multiplier=0)
            iota_h = constp.tile([128, maxnch * WSZ], mybir.dt.float16)
            nc.any.tensor_copy(out=iota_h[:], in_=iota_i[:])

            zero64_t = constp.tile([128, D], mybir.dt.float16)
            zrhs_t = constp.tile([128, RB], mybir.dt.float16)
            nc.vector.memset(zero64_t[:], 0.0)
            nc.vector.memset(zrhs_t[:], 0.0)


            for b in range(nb):
                c0 = int(prep["c_off"][b])
                nch = prep["block_nch"][b]
                if nch == 0:
                    continue
                Gq_t = gp.tile([128, nch, D], mybir.dt.int8, tag="Gq")
                for g0 in range(nch):
                    nc.gpsimd.indirect_dma_start(
                        out=Gq_t[:, g0, :],
                        out_offset=None,
                        in_=gathered[:],
                        in_offset=bass.IndirectOffsetOnAxis(
                            ap=idx_t[:, c0 + g0:c0 + g0 + 1], axis=0
                        ),
                    )
                G_t = gp.tile([128, nch, D], mybir.dt.float16, tag="G")
                nc.any.tensor_copy(out=G_t[:], in_=Gq_t[:])
                S01_t = sp.tile([128, nch * WSZ], mybir.dt.float16, tag="S01")
                S_t = sp.tile([128, nch * WSZ], mybir.dt.float16, tag="S")
                rl_b = rl_h[:, c0:c0 + nch, None].to_broadcast([128, nch, WSZ])
                vl_b = vs_h[:, c0:c0 + nch, None].to_broadcast([128, nch, WSZ])
                iota3 = iota_h[:, :nch * WSZ].rearrange("p (c j) -> p c j", c=nch)
                S01_3 = S01_t[:].rearrange("p (c j) -> p c j", c=nch)
                S_3 = S_t[:].rearrange("p (c j) -> p c j", c=nch)
                nc.vector.tensor_tensor(S01_3, iota3, rl_b, mybir.AluOpType.is_equal)
                nc.vector.tensor_tensor(S_3, S01_3, vl_b, mybir.AluOpType.mult)

                psum_t = pp.tile([D, RB], mybir.dt.float32)
                nc.tensor.matmul(
                    out=psum_t[:], lhsT=zero64_t[:], rhs=zrhs_t[:],
                    start=True, stop=False,
                )
                for ci in range(nch):
                    w = w_list[prep["block_windows"][b][ci]]
                    nc.tensor.matmul(
                        out=psum_t[:, w:w + WSZ],
                        lhsT=G_t[:, ci, :],
                        rhs=S_t[:, ci * WSZ:(ci + 1) * WSZ],
                        start=False,
                        stop=(ci == nch - 1),
                    )
                cw = min(RB, rpc - b * RB)  # last block is partial
                # per-block int8 quantization: s = absmax(psum); q = psum*126/s
                red_t = stp.tile([D, 1], mybir.dt.float32, tag="red")
                nc.vector.tensor_reduce(
                    out=red_t[:], in_=psum_t[:], axis=mybir.AxisListType.XYZW,
                    op=mybir.AluOpType.max, apply_absolute_value=True,
                )
                s_t = stp.tile([D, 1], mybir.dt.float32, tag="s")
                nc.gpsimd.partition_all_reduce(
                    s_t[:], red_t[:], channels=D, reduce_op=bass_isa.ReduceOp.max
                )
                nc.vector.tensor_scalar(
                    s_t[:], s_t[:], 1e-30, None, mybir.AluOpType.max
                )
                rcp_t = stp.tile([D, 1], mybir.dt.float32, tag="rcp")
                nc.vector.reciprocal(rcp_t[:], s_t[:])
                stage_t = stp.tile([D, RB], mybir.dt.int8, tag="stage")
                nc.vector.tensor_scalar(
                    stage_t[:], psum_t[:], rcp_t[:], 124.0,
                    mybir.AluOpType.mult, mybir.AluOpType.mult,
                )
                nc.sync.dma_start(
                    outT_d[:, b * RB:b * RB + cw], stage_t[:, :cw]
                )
                nc.sync.dma_start(oscale_d[:, b:b + 1], rcp_t[0:1, :])

    nc.finalize()
    return nc


def _make_in_maps(prep):
    n = prep["n_nodes"]
    nsh = n // N_CORES
    eq = prep["eq"]
    return [
        dict(
            eshard=np.ascontiguousarray(eq[k * nsh:(k + 1) * nsh]),
            packed=prep["packed"][k],
        )
        for k in range(N_CORES)
    ]


def _make_executor(nc):
    """Compile ``nc`` for the 8 axon-tunneled cores.

    Mirrors ``concourse.bass2jax.run_bass_via_pjrt`` with two changes: the
    operand slots for kernel outputs receive 1-byte dummies instead of
    host-transferred full-size zero buffers (the NEFF never binds those
    operands -- outputs go to the custom-call results -- so this is valid
    because this kernel writes every element of its outputs), and the jitted
    callable is returned so repeat calls skip retracing.
    """
    import jax
    from jax.experimental.shard_map import shard_map
    from jax.sharding import Mesh, PartitionSpec

    import concourse.mybir as mybir
    from concourse import bass2jax

    bass2jax.install_neuronx_cc_hook()
    partition_name = (
        nc.partition_id_tensor.name if nc.partition_id_tensor else None
    )
    in_names, out_names, out_avals = [], [], []
    for alloc in nc.m.functions[0].allocations:
        if not isinstance(alloc, mybir.MemoryLocationSet):
            continue
        name = alloc.memorylocations[0].name
        if alloc.kind == "ExternalInput":
            if name != partition_name:
                in_names.append(name)
        elif alloc.kind == "ExternalOutput":
            out_names.append(name)
            out_avals.append(
                jax.core.ShapedArray(
                    tuple(alloc.tensor_shape), mybir.dt.np(alloc.dtype)
                )
            )
    n_params = len(in_names)
    all_names = list(in_names) + list(out_names)
    if partition_name is not None:
        all_names.append(partition_name)

    def _body(*args):
        operands = list(args)
        if partition_name is not None:
            operands.append(bass2jax.partition_id_tensor())
        outs = bass2jax._bass_exec_p.bind(
            *operands,
            out_avals=tuple(out_avals),
            in_names=tuple(all_names),
            out_names=tuple(out_names),
            lowering_input_output_aliases=(),
            sim_require_finite=True,
            sim_require_nnan=True,
            nc=nc,
        )
        return tuple(outs)

    devices = jax.devices()[:N_CORES]
    assert len(devices) == N_CORES
    mesh = Mesh(np.asarray(devices), ("core",))
    n_args = n_params + len(out_names)  # output slots get 1-byte dummies
    jitted = jax.jit(
        shard_map(
            _body,
            mesh=mesh,
            in_specs=(PartitionSpec("core"),) * n_args,
            out_specs=(PartitionSpec("core"),) * len(out_names),
            check_rep=False,
        ),
        keep_unused=True,
    )
    return dict(jitted=jitted, in_names=in_names, out_names=out_names,
                out_avals=out_avals)


def _concat_inputs(ex, in_maps):
    cat = [
        np.concatenate([np.asarray(m[name]) for m in in_maps], axis=0)
        for name in ex["in_names"]
    ]
    cat.extend(np.zeros(N_CORES, np.int8) for _ in ex["out_names"])
    return cat


def _execute(ex, concat_in):
    """One full device execution: H2D transfers, kernel, D2H transfers."""
    out_arrs = ex["jitted"](*concat_in)
    for a in out_arrs:
        try:
            a.copy_to_host_async()
        except Exception:
            pass
    return [np.asarray(a) for a in out_arrs]


def _assemble(prep, ex, outs):
    n = prep["n_nodes"]
    rpc, nb = prep["rpc"], prep["nb"]
    outT = outs[ex["out_names"].index("outT")].reshape(N_CORES, D, rpc)
    osc = outs[ex["out_names"].index("oscale")].reshape(N_CORES, nb)
    out = np.empty((n, D), np.float32)
    for k in range(N_CORES):
        # oscale holds the device's 1/s; dividing by it exactly cancels any
        # reciprocal approximation error
        colscale = np.repeat(1.0 / (osc[k] * 124.0), RB)[:rpc].astype(np.float32)
        out[k * rpc:(k + 1) * rpc] = (
            outT[k].astype(np.float32) * colscale[None, :]
        ).T
    out *= prep["out_scale"]
    return out


def _validate_sample(out, edge_row, edge_col, edge_val, embeds, n):
    """Cheap integrity check: exact segment-sums for a few sampled rows.

    Catches the rare silent device failure (all-zero / garbage output from a
    wedged first execution) -- not a precision check, so the tolerance is
    loose relative to the int8 quantization error.
    """
    rows = np.linspace(0, n - 1, 24).astype(np.int64)
    lo = np.searchsorted(edge_row, rows, side="left")
    hi = np.searchsorted(edge_row, rows, side="right")
    emb = np.asarray(embeds, np.float64)
    worst = 0.0
    scale = 0.0
    for r, a, b in zip(rows, lo, hi):
        if a == b:
            continue
        exp = (
            np.asarray(edge_val[a:b], np.float64)[:, None] * emb[edge_col[a:b]]
        ).sum(axis=0)
        worst = max(worst, float(np.abs(out[r] - exp).max()))
        scale = max(scale, float(np.abs(exp).max()))
    return worst <= 0.05 * max(scale, 1.0)


def _kernel_impl(edge_row, edge_col, edge_val, embeds, num_nodes, attempts=3):
    n = int(num_nodes)
    edge_row = np.asarray(edge_row)
    edge_col = np.asarray(edge_col)
    edge_val = np.asarray(edge_val)

    prep = _prepare(edge_row, edge_col, edge_val, embeds, n)
    nc = _build_program(prep)
    ex = _make_executor(nc)
    concat_in = _concat_inputs(ex, _make_in_maps(prep))
    out = None
    for _ in range(attempts):
        outs = _execute(ex, concat_in)
        out = _assemble(prep, ex, outs)
        if _validate_sample(out, edge_row, edge_col, edge_val, embeds, n):
            return out
    raise RuntimeError("device output failed sample validation")


def _child_main(in_path, out_path):
    data = np.load(in_path)
    out = _kernel_impl(
        data["edge_row"], data["edge_col"], data["edge_val"],
        data["embeds"], int(data["num_nodes"]),
    )
    tmp = out_path + ".tmp.npy"
    np.save(tmp, out)
    os.replace(tmp, out_path)


def kernel(edge_row, edge_col, edge_val, embeds, num_nodes):
    """Run the device work in a subprocess with timeout + retry.

    A first execution on the axon-tunneled cores very occasionally wedges
    (collective setup race on the terminal); killing the client process
    releases it and a fresh process succeeds. Subprocess isolation turns
    that rare hang into a retry instead of an indefinite stall.
    """
    import subprocess
    import sys
    import tempfile

    here = os.path.dirname(os.path.abspath(__file__))
    with tempfile.TemporaryDirectory() as td:
        in_path = os.path.join(td, "in.npz")
        out_path = os.path.join(td, "out.npy")
        np.savez(
            in_path,
            edge_row=np.asarray(edge_row),
            edge_col=np.asarray(edge_col),
            edge_val=np.asarray(edge_val),
            embeds=np.asarray(embeds),
            num_nodes=np.asarray(int(num_nodes)),
        )
        code = (
            "import sys; sys.path.insert(0, sys.argv[1]); "
            "import kernel; kernel._child_main(sys.argv[2], sys.argv[3])"
        )
        last_err = None
        for timeout in (480, 480, 900):
            try:
                r = subprocess.run(
                    [sys.executable, "-c", code, here, in_path, out_path],
                    timeout=timeout,
                )
                if r.returncode == 0 and os.path.exists(out_path):
                    out = np.load(out_path)
                    if _validate_sample(
                        out, np.asarray(edge_row), np.asarray(edge_col),
                        np.asarray(edge_val), embeds, int(num_nodes),
                    ):
                        return out
                    last_err = RuntimeError("child output failed validation")
                    os.remove(out_path)
                else:
                    last_err = RuntimeError(f"child exited rc={r.returncode}")
            except subprocess.TimeoutExpired as e:
                last_err = e
        # final attempt in-process
        try:
            return _kernel_impl(
                edge_row, edge_col, edge_val, embeds, num_nodes, attempts=2
            )
        except Exception:
            raise last_err


# revision 36
# speedup vs baseline: 23.3411x; 1.0191x over previous
"""GCN layer (out = A @ embeds, A in sorted-row COO) on 8 Trainium2 cores.

Strategy (row-partitioned SpMM, wire-optimized):
  - Shard output rows across 8 cores (12500 rows each); each core owns the
    contiguous edge range whose destination rows fall in its slice (edge_row
    is sorted).
  - The embeds table is NOT replicated over the (slow) host link: each core
    receives only its 1/8 row-shard in fp16 and the full table is rebuilt
    on-device with an HBM AllGather over NeuronLink.
  - Each edge travels as ONE int32: col index (17 bits) | row offset within
    its 16-row window (4 bits) | value quantized to 11 bits. The value scale
    is folded into the fp16 embeds on the host. Unpacking (shift/and,
    int->fp16 convert) happens on-device on the DVE.
  - Per core, output rows are processed in blocks of 512; the block
    accumulator lives in PSUM as outT [64 (D, partitions), 512 (rows, free)].
  - Edges are packed on the host into chunks of 128 (one SBUF partition per
    edge). Each chunk is assigned a 16-row window on a stride-7 grid within
    its block; windows are merged across cores so one SPMD program serves
    all 8 cores (data-dependent structure lives in the input arrays).
  - Per chunk: an indirect DMA gathers the 128 referenced fp16 embed rows
    (G [128, 64]); a selection matrix S [128, 16] with
    S[p, j] = val[p] * (rowloc[p] == j) is built by two DVE tensor_tensor
    ops against an on-device iota; one fp16 matmul accumulates G^T @ S into
    the psum window (rows on the free axis, so window offsets are
    unconstrained).
  - Blocks are flushed psum -> SBUF (fp16) -> DRAM as outT [64, 12500] per
    core; the final transpose/concat happens on the host.
"""
import math
import os

import numpy as np

WSZ = 16
WSTRIDE = 7
N_CORES = 8
RB = 512
D = 64
VQ_MAX = 2047  # 11-bit value quantization


def _build_windows(rb):
    ws = list(range(0, rb - WSZ + 1, WSTRIDE))
    if ws[-1] != rb - WSZ:
        ws.append(rb - WSZ)
    return ws


def _pack_core(edge_row, rptr, r0, r1, rb, w_list):
    nb = math.ceil((r1 - r0) / rb)
    nwin = len(w_list)
    last_w = w_list[-1]
    grid_last = (nwin - 1) * WSTRIDE
    blocks = []
    for b in range(nb):
        br0 = r0 + b * rb
        br1 = min(br0 + rb, r1)
        win_chunks = {}
        for r in range(br0, br1):
            s, e = rptr[r], rptr[r + 1]
            if s == e:
                continue
            rl = r - br0
            jlo = max(0, -(-(rl - (WSZ - 1)) // WSTRIDE))
            jhi = min(rl // WSTRIDE, nwin - 1)
            elig = list(range(jlo, jhi + 1))
            if last_w != grid_last and last_w <= rl <= last_w + WSZ - 1:
                if not (elig and elig[-1] == nwin - 1):
                    elig.append(nwin - 1)
            elig = [j for j in elig if w_list[j] <= rl <= w_list[j] + WSZ - 1]
            rem = e - s
            pos = s
            for j in elig:
                if rem == 0:
                    break
                for ch in win_chunks.get(j, []):
                    space = 128 - ch[0]
                    if space <= 0:
                        continue
                    take = min(space, rem)
                    ch[1].append((pos, take, rl - w_list[j]))
                    ch[0] += take
                    pos += take
                    rem -= take
                    if rem == 0:
                        break
            while rem > 0:
                j = elig[-1]
                take = min(128, rem)
                win_chunks.setdefault(j, []).append(
                    [take, [(pos, take, rl - w_list[j])]]
                )
                pos += take
                rem -= take
        blocks.append(win_chunks)
    return blocks


def _prepare(edge_row, edge_col, edge_val, embeds, n_nodes):
    assert n_nodes < (1 << 17) and n_nodes % N_CORES == 0
    rpc = n_nodes // N_CORES
    nb = math.ceil(rpc / RB)
    w_list = _build_windows(RB)
    nwin = len(w_list)
    rptr = np.searchsorted(edge_row, np.arange(n_nodes + 1)).astype(np.int64)

    per_core = [
        _pack_core(edge_row, rptr, k * rpc, (k + 1) * rpc, RB, w_list)
        for k in range(N_CORES)
    ]

    nch = np.zeros((nb, nwin), np.int64)
    for k in range(N_CORES):
        for b in range(nb):
            for j, chs in per_core[k][b].items():
                nch[b, j] = max(nch[b, j], len(chs))

    block_windows = []
    block_nch = []
    for b in range(nb):
        lst = []
        for j in range(nwin):
            lst.extend([j] * int(nch[b, j]))
        block_windows.append(lst)
        block_nch.append(len(lst))
    c_off = np.concatenate([[0], np.cumsum(block_nch)]).astype(np.int64)
    totch = int(c_off[-1])

    # int8 embeds with per-row scales: eq[r, d] = round(e[r, d] * 127 / rs[r]).
    # The per-row scale is folded into the 11-bit value quantization on the
    # host (host knows each edge's column), so the device just computes
    # sum (vq/2047) * eq and the host multiplies by wscale/127 afterwards.
    emb = np.asarray(embeds, np.float64)
    rs = np.maximum(np.abs(emb).max(axis=1), 1e-30)  # [n_nodes]
    eq = np.rint(emb / rs[:, None] * 127.0).astype(np.int8)
    w_all = np.asarray(edge_val, np.float64) * rs[edge_col]
    wscale = max(float(w_all.max()) if w_all.size else 1.0, 1e-30)
    vq_all = np.clip(np.rint(w_all / wscale * VQ_MAX), 0, VQ_MAX).astype(np.int64)

    # packed word: idx | rowloc << 17 | vq << 21  (pad: idx=0, rl=0, vq=0)
    packed = np.zeros((N_CORES, 128, totch), np.int32)
    for k in range(N_CORES):
        for b in range(nb):
            slot_of = {}
            cnt = {}
            for ci, j in enumerate(block_windows[b]):
                slot_of[(j, cnt.get(j, 0))] = int(c_off[b]) + ci
                cnt[j] = cnt.get(j, 0) + 1
            for j, chs in per_core[k][b].items():
                for copy, ch in enumerate(chs):
                    c = slot_of[(j, copy)]
                    p = 0
                    for (pos, take, rl) in ch[1]:
                        word = (
                            edge_col[pos:pos + take].astype(np.int64)
                            | (rl << 17)
                            | (vq_all[pos:pos + take] << 21)
                        )
                        packed[k, p:p + take, c] = word.astype(np.int32)
                        p += take

    maxnch = max(block_nch)
    return dict(
        n_nodes=n_nodes, nb=nb, rpc=rpc, w_list=w_list,
        block_windows=block_windows, block_nch=block_nch,
        c_off=c_off, totch=totch, maxnch=maxnch,
        packed=packed, eq=eq, out_scale=wscale / 127.0,
    )


def _build_program(prep):
    import concourse.bacc as bacc
    import concourse.bass as bass
    import concourse.bass_isa as bass_isa
    import concourse.mybir as mybir
    import concourse.tile as tile

    n = prep["n_nodes"]
    nb = prep["nb"]
    rpc = prep["rpc"]
    nsh = n // N_CORES
    totch, maxnch = prep["totch"], prep["maxnch"]
    w_list = prep["w_list"]

    nc = bacc.Bacc(
        "TRN2", target_bir_lowering=False, debug=False, num_devices=N_CORES
    )
    eshard_d = nc.dram_tensor(
        "eshard", [nsh, D], mybir.dt.int8, kind="ExternalInput"
    )
    packed_d = nc.dram_tensor(
        "packed", [128, totch], mybir.dt.int32, kind="ExternalInput"
    )
    outT_d = nc.dram_tensor("outT", [D, rpc], mybir.dt.int8, kind="ExternalOutput")
    oscale_d = nc.dram_tensor("oscale", [1, nb], mybir.dt.float32, kind="ExternalOutput")

    with tile.TileContext(nc) as tc:
        with (
            tc.tile_pool(name="dram", bufs=1, space="DRAM") as dram,
            tc.tile_pool(name="const", bufs=1) as constp,
            tc.tile_pool(name="gp", bufs=2) as gp,
            tc.tile_pool(name="sp", bufs=2) as sp,
            tc.tile_pool(name="stp", bufs=2) as stp,
            tc.tile_pool(name="pp", bufs=2, space="PSUM") as pp,
        ):
            bounce = dram.tile([nsh, D], mybir.dt.int8)
            gathered = dram.tile([n, D], mybir.dt.int8)
            nc.gpsimd.dma_start(bounce[:], eshard_d[:])
            nc.gpsimd.collective_compute(
                "AllGather",
                mybir.AluOpType.bypass,
                replica_groups=[list(range(N_CORES))],
                ins=[bounce.opt()],
                outs=[gathered.opt()],
            )

            packed_t = constp.tile([128, totch], mybir.dt.int32)
            nc.sync.dma_start(packed_t[:], packed_d[:])
            idx_t = constp.tile([128, totch], mybir.dt.int32)
            rl_h = constp.tile([128, totch], mybir.dt.float16)
            vs_h = constp.tile([128, totch], mybir.dt.float16)
            tmp_i = constp.tile([128, totch], mybir.dt.int32)
            nc.vector.tensor_scalar(
                idx_t[:], packed_t[:], 0x1FFFF, None, mybir.AluOpType.bitwise_and
            )
            nc.vector.tensor_scalar(
                tmp_i[:], packed_t[:], 17, 0xF,
                mybir.AluOpType.logical_shift_right, mybir.AluOpType.bitwise_and,
            )
            nc.any.tensor_copy(out=rl_h[:], in_=tmp_i[:])
            nc.vector.tensor_scalar(
                tmp_i[:], packed_t[:], 21, None, mybir.AluOpType.logical_shift_right
            )
            nc.any.tensor_copy(out=vs_h[:], in_=tmp_i[:])
            nc.vector.tensor_scalar(
                vs_h[:], vs_h[:], 1.0 / VQ_MAX, None, mybir.AluOpType.mult
            )

            iota_i = constp.tile([128, maxnch * WSZ], mybir.dt.int32)
            nc.gpsimd.iota(iota_i[:], [[0, maxnch], [1, WSZ]], channel_multiplier=0)
            iota_h = constp.tile([128, maxnch * WSZ], mybir.dt.float16)
            nc.any.tensor_copy(out=iota_h[:], in_=iota_i[:])

            zero64_t = constp.tile([128, D], mybir.dt.float16)
            zrhs_t = constp.tile([128, RB], mybir.dt.float16)
            nc.vector.memset(zero64_t[:], 0.0)
            nc.vector.memset(zrhs_t[:], 0.0)


            for b in range(nb):
                c0 = int(prep["c_off"][b])
                nch = prep["block_nch"][b]
                if nch == 0:
                    continue
                Gq_t = gp.tile([128, nch, D], mybir.dt.int8, tag="Gq")
                for g0 in range(nch):
                    nc.gpsimd.indirect_dma_start(
                        out=Gq_t[:, g0, :],
                        out_offset=None,
                        in_=gathered[:],
                        in_offset=bass.IndirectOffsetOnAxis(
                            ap=idx_t[:, c0 + g0:c0 + g0 + 1], axis=0
                        ),
                    )
                G_t = gp.tile([128, nch, D], mybir.dt.float16, tag="G")
                nc.any.tensor_copy(out=G_t[:], in_=Gq_t[:])
                S01_t = sp.tile([128, nch * WSZ], mybir.dt.float16, tag="S01")
                S_t = sp.tile([128, nch * WSZ], mybir.dt.float16, tag="S")
                rl_b = rl_h[:, c0:c0 + nch, None].to_broadcast([128, nch, WSZ])
                vl_b = vs_h[:, c0:c0 + nch, None].to_broadcast([128, nch, WSZ])
                iota3 = iota_h[:, :nch * WSZ].rearrange("p (c j) -> p c j", c=nch)
                S01_3 = S01_t[:].rearrange("p (c j) -> p c j", c=nch)
                S_3 = S_t[:].rearrange("p (c j) -> p c j", c=nch)
                nc.vector.tensor_tensor(S01_3, iota3, rl_b, mybir.AluOpType.is_equal)
                nc.vector.tensor_tensor(S_3, S01_3, vl_b, mybir.AluOpType.mult)

                psum_t = pp.tile([D, RB], mybir.dt.float32)
                nc.tensor.matmul(
                    out=psum_t[:], lhsT=zero64_t[:], rhs=zrhs_t[:],
                    start=True, stop=False,
                )
                for ci in range(nch):
                    w = w_list[prep["block_windows"][b][ci]]
                    nc.tensor.matmul(
                        out=psum_t[:, w:w + WSZ],
                        lhsT=G_t[:, ci, :],
                        rhs=S_t[:, ci * WSZ:(ci + 1) * WSZ],
                        start=False,
                        stop=(ci == nch - 1),
                    )
                cw = min(RB, rpc - b * RB)  # last block is partial
                # per-block int8 quantization: s = absmax(psum); q = psum*126/s
                red_t = stp.tile([D, 1], mybir.dt.float32, tag="red")
                nc.vector.tensor_reduce(
                    out=red_t[:], in_=psum_t[:], axis=mybir.AxisListType.XYZW,
                    op=mybir.AluOpType.max, apply_absolute_value=True,
                )
                s_t = stp.tile([D, 1], mybir.dt.float32, tag="s")
                nc.gpsimd.partition_all_reduce(
                    s_t[:], red_t[:], channels=D, reduce_op=bass_isa.ReduceOp.max
                )
                nc.vector.tensor_scalar(
                    s_t[:], s_t[:], 1e-30, None, mybir.AluOpType.max
                )
                rcp_t = stp.tile([D, 1], mybir.dt.float32, tag="rcp")
                nc.vector.reciprocal(rcp_t[:], s_t[:])
                stage_t = stp.tile([D, RB], mybir.dt.int8, tag="stage")
                nc.vector.tensor_scalar(
                    stage_t[:], psum_t[:], rcp_t[:], 124.0,
                    mybir.AluOpType.mult, mybir.AluOpType.mult,
                )
                nc.sync.dma_start(
                    outT_d[:, b * RB:b * RB + cw], stage_t[:, :cw]
                )
                nc.sync.dma_start(oscale_d[:, b:b + 1], rcp_t[0:1, :])

    nc.finalize()
    return nc


def _make_in_maps(prep):
    n = prep["n_nodes"]
    nsh = n // N_CORES
    eq = prep["eq"]
    return [
        dict(
            eshard=np.ascontiguousarray(eq[k * nsh:(k + 1) * nsh]),
            packed=prep["packed"][k],
        )
        for k in range(N_CORES)
    ]


def _make_executor(nc):
    """Compile ``nc`` for the 8 axon-tunneled cores.

    Mirrors ``concourse.bass2jax.run_bass_via_pjrt`` with two changes: the
    operand slots for kernel outputs receive 1-byte dummies instead of
    host-transferred full-size zero buffers (the NEFF never binds those
    operands -- outputs go to the custom-call results -- so this is valid
    because this kernel writes every element of its outputs), and the jitted
    callable is returned so repeat calls skip retracing.
    """
    import jax
    from jax.experimental.shard_map import shard_map
    from jax.sharding import Mesh, PartitionSpec

    import concourse.mybir as mybir
    from concourse import bass2jax

    bass2jax.install_neuronx_cc_hook()
    partition_name = (
        nc.partition_id_tensor.name if nc.partition_id_tensor else None
    )
    in_names, out_names, out_avals = [], [], []
    for alloc in nc.m.functions[0].allocations:
        if not isinstance(alloc, mybir.MemoryLocationSet):
            continue
        name = alloc.memorylocations[0].name
        if alloc.kind == "ExternalInput":
            if name != partition_name:
                in_names.append(name)
        elif alloc.kind == "ExternalOutput":
            out_names.append(name)
            out_avals.append(
                jax.core.ShapedArray(
                    tuple(alloc.tensor_shape), mybir.dt.np(alloc.dtype)
                )
            )
    n_params = len(in_names)
    all_names = list(in_names) + list(out_names)
    if partition_name is not None:
        all_names.append(partition_name)

    def _body(*args):
        operands = list(args)
        if partition_name is not None:
            operands.append(bass2jax.partition_id_tensor())
        outs = bass2jax._bass_exec_p.bind(
            *operands,
            out_avals=tuple(out_avals),
            in_names=tuple(all_names),
            out_names=tuple(out_names),
            lowering_input_output_aliases=(),
            sim_require_finite=True,
            sim_require_nnan=True,
            nc=nc,
        )
        return tuple(outs)

    devices = jax.devices()[:N_CORES]
    assert len(devices) == N_CORES
    mesh = Mesh(np.asarray(devices), ("core",))
    n_args = n_params + len(out_names)  # output slots get 1-byte dummies
    jitted = jax.jit(
        shard_map(
            _body,
            mesh=mesh,
            in_specs=(PartitionSpec("core"),) * n_args,
            out_specs=(PartitionSpec("core"),) * len(out_names),
            check_rep=False,
        ),
        keep_unused=True,
    )
    return dict(jitted=jitted, in_names=in_names, out_names=out_names,
                out_avals=out_avals)


def _concat_inputs(ex, in_maps):
    cat = [
        np.concatenate([np.asarray(m[name]) for m in in_maps], axis=0)
        for name in ex["in_names"]
    ]
    cat.extend(np.zeros(N_CORES, np.int8) for _ in ex["out_names"])
    return cat


def _execute(ex, concat_in):
    """One full device execution: H2D transfers, kernel, D2H transfers."""
    out_arrs = ex["jitted"](*concat_in)
    for a in out_arrs:
        try:
            a.copy_to_host_async()
        except Exception:
            pass
    return [np.asarray(a) for a in out_arrs]


def _assemble(prep, ex, outs):
    n = prep["n_nodes"]
    rpc, nb = prep["rpc"], prep["nb"]
    outT = outs[ex["out_names"].index("outT")].reshape(N_CORES, D, rpc)
    osc = outs[ex["out_names"].index("oscale")].reshape(N_CORES, nb)
    out = np.empty((n, D), np.float32)
    for k in range(N_CORES):
        # oscale holds the device's 1/s; dividing by it exactly cancels any
        # reciprocal approximation error
        colscale = np.repeat(1.0 / (osc[k] * 124.0), RB)[:rpc].astype(np.float32)
        out[k * rpc:(k + 1) * rpc] = (
            outT[k].astype(np.float32) * colscale[None, :]
        ).T
    out *= prep["out_scale"]
    return out


def _validate_sample(out, edge_row, edge_col, edge_val, embeds, n):
    """Cheap integrity check: exact segment-sums for a few sampled rows.

    Catches the rare silent device failure (all-zero / garbage output from a
    wedged first execution) -- not a precision check, so the tolerance is
    loose relative to the int8 quantization error.
    """
    rows = np.linspace(0, n - 1, 24).astype(np.int64)
    lo = np.searchsorted(edge_row, rows, side="left")
    hi = np.searchsorted(edge_row, rows, side="right")
    emb = np.asarray(embeds, np.float64)
    worst = 0.0
    scale = 0.0
    for r, a, b in zip(rows, lo, hi):
        if a == b:
            continue
        exp = (
            np.asarray(edge_val[a:b], np.float64)[:, None] * emb[edge_col[a:b]]
        ).sum(axis=0)
        worst = max(worst, float(np.abs(out[r] - exp).max()))
        scale = max(scale, float(np.abs(exp).max()))
    return worst <= 0.05 * max(scale, 1.0)


def _kernel_impl(edge_row, edge_col, edge_val, embeds, num_nodes, attempts=3):
    n = int(num_nodes)
    edge_row = np.asarray(edge_row)
    edge_col = np.asarray(edge_col)
    edge_val = np.asarray(edge_val)

    prep = _prepare(edge_row, edge_col, edge_val, embeds, n)
    nc = _build_program(prep)
    ex = _make_executor(nc)
    concat_in = _concat_inputs(ex, _make_in_maps(prep))
    out = None
    for _ in range(attempts):
        outs = _execute(ex, concat_in)
        out = _assemble(prep, ex, outs)
        if _validate_sample(out, edge_row, edge_col, edge_val, embeds, n):
            return out
    raise RuntimeError("device output failed sample validation")


def _child_main(in_path, out_path):
    data = np.load(in_path)
    out = _kernel_impl(
        data["edge_row"], data["edge_col"], data["edge_val"],
        data["embeds"], int(data["num_nodes"]),
    )
    tmp = out_path + ".tmp.npy"
    np.save(tmp, out)
    os.replace(tmp, out_path)


def kernel(edge_row, edge_col, edge_val, embeds, num_nodes):
    """Run the device work in a subprocess with timeout + retry.

    A first execution on the axon-tunneled cores very occasionally wedges
    (collective setup race on the terminal); killing the client process
    releases it and a fresh process succeeds. Subprocess isolation turns
    that rare hang into a retry instead of an indefinite stall.
    """
    import subprocess
    import sys
    import tempfile

    here = os.path.dirname(os.path.abspath(__file__))
    with tempfile.TemporaryDirectory() as td:
        in_path = os.path.join(td, "in.npz")
        out_path = os.path.join(td, "out.npy")
        np.savez(
            in_path,
            edge_row=np.asarray(edge_row),
            edge_col=np.asarray(edge_col),
            edge_val=np.asarray(edge_val),
            embeds=np.asarray(embeds),
            num_nodes=np.asarray(int(num_nodes)),
        )
        code = (
            "import sys; sys.path.insert(0, sys.argv[1]); "
            "import kernel; kernel._child_main(sys.argv[2], sys.argv[3])"
        )
        last_err = None
        for timeout in (480, 480, 900):
            try:
                r = subprocess.run(
                    [sys.executable, "-c", code, here, in_path, out_path],
                    timeout=timeout,
                )
                if r.returncode == 0 and os.path.exists(out_path):
                    out = np.load(out_path)
                    if _validate_sample(
                        out, np.asarray(edge_row), np.asarray(edge_col),
                        np.asarray(edge_val), embeds, int(num_nodes),
                    ):
                        return out
                    last_err = RuntimeError("child output failed validation")
                    os.remove(out_path)
                else:
                    last_err = RuntimeError(f"child exited rc={r.returncode}")
            except subprocess.TimeoutExpired as e:
                last_err = e
        # final attempt in-process
        try:
            return _kernel_impl(
                edge_row, edge_col, edge_val, embeds, num_nodes, attempts=2
            )
        except Exception:
            raise last_err
